# revision 14
# baseline (speedup 1.0000x reference)
"""Multi-head causal attention on 8 TRN2 NeuronCores.

B=2, S=2048, D=1024, H=16 heads, head_dim=64. Tensor-parallel over heads:
core c owns heads {2c, 2c+1}. Each core:
  stage 1 (per 512-token piece): qT/kT/vT = W_c @ x.T (feature-major,
           bf16 matmuls, fp32 psum), then v -> token-major via PE
           transpose with a ones column per head appended (gives the
           softmax denominator for free).
  stage 2: attention in scoresT (k-major) layout, joint over the two
           heads: for each 128-wide k-block j, both heads' score
           matmuls (64-deep contractions at PE row groups 0 and 64)
           are issued back-to-back into one shared [128,1024] PSUM
           tile so the hardware runs them concurrently.  One ScalarE
           exp covers both heads; causal handling is exact per block:
           fully-masked q-columns are never computed, and the single
           128-wide triangular window is masked with one small
           multiply (alternating VectorE/GpSimd).  ctxT' accumulation
           = [v|1].T @ pT per head; normalization by the ones-row sum
           via reciprocal_approx_fast + partition broadcast.
  stage 3: partial output projection split per head into 64-deep
           contractions at row groups 0/64 so consecutive f-blocks
           overlap on the PE; woven into the round stream.
A short burst of dummy matmuls at t=0 warms the PE HAM clock gate
during the initial x-DMA wait.  Host sums the 8 partial outputs and
adds the bias.
"""
import numpy as np
import ml_dtypes

B, S, D, H = 2, 2048, 1024, 16
HD = 64          # head dim
NT = B * S       # 4096 tokens
P = 128          # partitions
NCORES = 8
HPC = 2          # heads per core
NM = S // 512    # 4 q-pieces per batch
NP = NT // 512   # 8 token pieces overall
VCB = 2 * (HD + 1)   # 130: v block cols: h0 feats+1, h1 feats+1

_cache = {}


def _build():
    import concourse.bass as bass
    import concourse.mybir as mybir
    from concourse import bacc
    import concourse.tile as tile
    from concourse.masks import make_identity

    BF16 = mybir.dt.bfloat16
    F32 = mybir.dt.float32
    Exp = mybir.ActivationFunctionType.Exp

    nc = bacc.Bacc("TRN2", target_bir_lowering=False, debug=False,
                   num_devices=NCORES)

    xT_d = nc.dram_tensor("xT", [D, NT], BF16, kind="ExternalInput")
    wq_d = nc.dram_tensor("wq", [P, D], BF16, kind="ExternalInput")
    wk_d = nc.dram_tensor("wk", [P, D], BF16, kind="ExternalInput")
    wv_d = nc.dram_tensor("wv", [P, D], BF16, kind="ExternalInput")
    wo_d = nc.dram_tensor("wo", [P, D], BF16, kind="ExternalInput")
    mask_d = nc.dram_tensor("mask", [P, 256], BF16, kind="ExternalInput")
    out_d = nc.dram_tensor("out", [D, NT], BF16, kind="ExternalOutput")

    with tile.TileContext(nc) as tc:
        with tc.tile_pool(name="const", bufs=1) as const, \
             tc.tile_pool(name="xp", bufs=1) as xp, \
             tc.tile_pool(name="qk", bufs=1) as qk, \
             tc.tile_pool(name="misc", bufs=4) as misc, \
             tc.tile_pool(name="stg", bufs=6) as stg, \
             tc.tile_pool(name="pt", bufs=8) as ptp, \
             tc.tile_pool(name="pp", bufs=2, space="PSUM") as pp, \
             tc.tile_pool(name="sc", bufs=2, space="PSUM") as scp, \
             tc.tile_pool(name="cx", bufs=2, space="PSUM") as cxp:

            # ---- constants / weights ----
            # x piece 0 first so stage 1 can begin ASAP; wq/wk/wv next;
            # wo + the mask are not needed until much later
            x_sb = [xp.tile([P, NT], BF16, tag=f"x{c}", name=f"x{c}")
                    for c in range(8)]
            # spread piece-0 chunk DMA issues across four engine queues:
            # a single queue issues one DMA descriptor per ~0.65us, which
            # would serialize the latency-critical first piece
            iss = [nc.sync, nc.sync, nc.sync, nc.scalar,
                   nc.scalar, nc.gpsimd, nc.gpsimd, nc.gpsimd]
            for c in range(8):
                iss[c].dma_start(x_sb[c][:, 0:512],
                                 xT_d.ap()[c * P:(c + 1) * P, 0:512])
            w_sb = {}
            for name, dd in (("wq", wq_d), ("wk", wk_d), ("wv", wv_d)):
                t = const.tile([P, D], BF16, tag=name)
                nc.sync.dma_start(t[:], dd.ap())
                w_sb[name] = t
            for n in range(1, NP):
                cols = slice(n * 512, (n + 1) * 512)
                for c in range(8):
                    nc.sync.dma_start(x_sb[c][:, cols],
                                      xT_d.ap()[c * P:(c + 1) * P, cols])
            wo = const.tile([P, D], BF16, tag="wo")
            nc.sync.dma_start(wo[:], wo_d.ap())
            w_sb["wo"] = wo
            mask_sb = const.tile([P, 256], BF16, tag="mask")
            nc.sync.dma_start(mask_sb[:], mask_d.ap())
            ident = const.tile([P, P], BF16, tag="ident")
            make_identity(nc, ident[:])

            # warm up the PE clock (the HAM throttle holds the PE at
            # half rate until it sees ~3.4us of sustained matmul
            # activity) while the first x DMAs are in flight
            warm = const.tile([P, 256], BF16, tag="warm")
            nc.vector.memset(warm[:], 0.0)
            wps = pp.tile([P, 512], F32, tag="p1", name="warmps")
            for i in range(40):
                nc.tensor.matmul(wps[:, 0:256], warm[:, 0:128], warm[:],
                                 start=True, stop=True)

            qT = qk.tile([P, NT], BF16, tag="qT")
            kT = qk.tile([P, NT], BF16, tag="kT")
            vT = qk.tile([P, NT], BF16, tag="vT")
            v_sb = qk.tile([P, (NT // P) * VCB], BF16, tag="v")
            nc.gpsimd.memset(v_sb[:], 1.0)
            ctxT = qk.tile([P, NT], BF16, tag="ctxT")

            # ---- stage 1 sub-units (half-size so they slot between
            # attention rounds without hogging the PE) ----
            s1_ps = {}

            def s1_proj_half(n, wname, dst, half):
                cols = slice(n * 512, (n + 1) * 512)
                w = w_sb[wname]
                if half == 0:
                    s1_ps[(wname, n)] = pp.tile([P, 512], F32, tag="p1",
                                                name=f"p1_{wname}_{n}")
                ps = s1_ps[(wname, n)]
                for cc in range(half * 4, half * 4 + 4):
                    nc.tensor.matmul(ps[:], w[:, cc * P:(cc + 1) * P],
                                     x_sb[cc][:, cols],
                                     start=(cc == 0), stop=(cc == 7))
                if half == 1:
                    # GpSimd cannot read PSUM; ScalarE is reserved for
                    # exp, so all stage-1 casts go to VectorE
                    nc.vector.tensor_copy(dst[:, cols], ps[:])
                    del s1_ps[(wname, n)]

            def s1_vtrans(n, half):
                # v -> token-major for 2 of the 4 blocks of this piece
                for t in range(4 * n + 2 * half, 4 * n + 2 * half + 2):
                    pst = pp.tile([P, P], BF16, tag="p1", name=f"ptr{t}")
                    nc.tensor.transpose(pst[:], vT[:, t * P:(t + 1) * P],
                                        ident[:])
                    # one 3D-AP copy places both heads' 64 feat cols
                    # (strides: head 65, feat 1), skipping the ones cols
                    dst3 = v_sb[:, t * VCB:(t + 1) * VCB].rearrange(
                        "p (h f) -> p h f", f=HD + 1)[:, :, 0:HD]
                    src3 = pst[:, :].rearrange("p (h f) -> p h f", f=HD)
                    nc.vector.tensor_copy(dst3, src3)

            q1 = []
            for n in range(NP):
                for wname, dst in (("wq", qT), ("wk", kT), ("wv", vT)):
                    for half in range(2):
                        q1.append((n, lambda n=n, w=wname, d=dst, h=half:
                                   s1_proj_half(n, w, d, h)))
                q1 += [(n, lambda n=n: s1_vtrans(n, 0)),
                       (n, lambda n=n: s1_vtrans(n, 1))]

            i1 = 0
            done1 = -1

            def pump_one():
                # exactly one stage-1 unit: issuing whole pieces between
                # rounds gives them scheduler priority over later
                # attention rounds and serializes the pipeline
                nonlocal i1, done1
                if i1 < len(q1):
                    n, fn = q1[i1]
                    fn()
                    if i1 + 1 >= len(q1) or q1[i1 + 1][0] != n:
                        done1 = n
                    i1 += 1

            def pump_q1(need):
                while done1 < need and i1 < len(q1):
                    pump_one()

            # ---- stage 3: partial output projection; staging casts on
            # VectorE with an occasional ScalarE one (GpSimd cannot
            # read PSUM) ----
            veng = [nc.vector, nc.vector, nc.vector, nc.scalar]
            s3_ctr = [0]

            def s3_quarter(n, qtr):
                cols = slice(n * 512, (n + 1) * 512)
                for f in range(qtr * 2, qtr * 2 + 2):
                    pso = pp.tile([P, 512], F32, tag="p1",
                                  name=f"p3_{f}_{n}")
                    nc.tensor.matmul(pso[:], wo[:, f * P:(f + 1) * P],
                                     ctxT[:, cols], start=True, stop=True)
                    st = stg.tile([P, 512], BF16, tag="st",
                                  name=f"st_{f}_{n}")
                    eng = veng[s3_ctr[0] % 4]
                    if eng is nc.scalar:
                        eng.copy(st[:], pso[:])
                    else:
                        eng.tensor_copy(st[:], pso[:])
                    s3_ctr[0] += 1
                    nc.sync.dma_start(
                        out_d.ap()[f * P:(f + 1) * P, cols], st[:])

            def normalize(b, m, hl, cx):
                hbase = hl * HD
                qc0 = b * S + m * 512
                sm = misc.tile([1, 512], F32, tag="sm",
                               name=f"sm_{b}_{m}_{hl}")
                nc.vector.tensor_copy(sm[:], cx[HD:HD + 1, :])
                rc = misc.tile([1, 512], F32, tag="rc",
                               name=f"rc_{b}_{m}_{hl}")
                nc.vector.reciprocal_approx_fast(rc[:], sm[:])
                bc = misc.tile([HD, 512], F32, tag="bc",
                               name=f"bc_{b}_{m}_{hl}")
                nc.gpsimd.partition_broadcast(bc[:], rc[:])
                nc.vector.tensor_mul(
                    ctxT[hbase:hbase + HD, qc0:qc0 + 512],
                    cx[0:HD, :], bc[:])

            # ---- main loop: per (b, m) group, per k-block rounds ----
            s3q = []
            rnd = 0
            meng = [nc.gpsimd, nc.gpsimd]

            for b in range(B):
                for m in range(NM):
                    pump_q1(b * NM + m)
                    njs = 4 * m + 4
                    qc0 = b * S + m * 512
                    cx = [cxp.tile([HD + 1, 512], F32, tag="cx",
                                   name=f"cx_{b}_{m}_{hl}")
                          for hl in range(HPC)]
                    for j in range(njs):
                        off = 128 * max(0, j - 4 * m)
                        kc0 = b * S + j * P
                        scs = scp.tile([P, 1024], F32, tag="sc",
                                       name=f"sc_{b}_{m}_{j}")
                        # both heads' score matmuls issued back-to-back:
                        # 64-deep contractions at PE row groups 0 / 64
                        # run concurrently
                        for hl in range(HPC):
                            hb = hl * HD
                            nc.tensor.matmul(
                                scs[:, hl * 512 + off:(hl + 1) * 512],
                                kT[hb:hb + HD, kc0:kc0 + P],
                                qT[hb:hb + HD, qc0 + off:qc0 + 512],
                                start=True, stop=True,
                                tile_position=(hb, 0))
                        # one joint exp over both heads' unmasked region
                        pt = ptp.tile([P, 1024], BF16, tag="pt",
                                      name=f"pt_{b}_{m}_{j}")
                        if off:
                            nc.scalar.activation(
                                pt[:].rearrange("p (a c) -> p a c",
                                                a=2)[:, :, off:512],
                                scs[:].rearrange("p (a c) -> p a c",
                                                 a=2)[:, :, off:512],
                                Exp, scale=0.125)
                        else:
                            nc.scalar.activation(pt[:], scs[:], Exp,
                                                 scale=0.125)
                        if j >= 4 * m:
                            # diagonal block: 128-wide triangular window
                            # at [off, off+128) in each half
                            pt3 = pt[:].rearrange(
                                "p (a c) -> p a c", a=2)[:, :, off:off + 128]
                            m3 = mask_sb[:, :].rearrange(
                                "p (a c) -> p a c", a=2)
                            meng[j % 2].tensor_mul(pt3, pt3, m3)
                        for hl in range(HPC):
                            vb = (b * (S // P) + j) * VCB + hl * (HD + 1)
                            nc.tensor.matmul(
                                cx[hl][:, off:512],
                                v_sb[:, vb:vb + HD + 1],
                                pt[:, hl * 512 + off:(hl + 1) * 512],
                                start=(j == 0), stop=(j == njs - 1))
                        # stage-1 / stage-3 filler keeps the PE queue fed
                        pump_one()
                        if s3q:
                            s3_quarter(*s3q.pop(0))
                        if len(s3q) > 1:
                            s3_quarter(*s3q.pop(0))
                        rnd += 1
                    for hl in range(HPC):
                        normalize(b, m, hl, cx[hl])
                    n = b * NM + m
                    s3q += [(n, 0), (n, 1), (n, 2), (n, 3)]
            pump_q1(NP)
            for n, qtr in s3q:
                s3_quarter(n, qtr)
    nc.compile()
    return nc


def _get_nc():
    if "nc" not in _cache:
        _cache["nc"] = _build()
    return _cache["nc"]


def _bf16(a):
    return np.ascontiguousarray(a).astype(ml_dtypes.bfloat16)


def _prepare_in_maps(x, Wq, Wk, Wv, Wo):
    xT = _bf16(np.asarray(x, np.float32).reshape(NT, D).T)
    # triangular window mask (p <= c), duplicated for the two halves
    pp_ = np.arange(P)[:, None]
    cc = np.arange(P)[None, :]
    tri = (pp_ <= cc).astype(np.float32)
    mask = _bf16(np.concatenate([tri, tri], axis=1))

    def wlayout(Wslice):  # [128 feats, 1024 d] -> [p, cc*128+f]
        return _bf16(Wslice.reshape(P, 8, P).transpose(2, 1, 0)
                     .reshape(P, D))

    in_maps = []
    for c in range(NCORES):
        rows = slice(c * P, (c + 1) * P)
        in_maps.append({
            "xT": xT,
            "wq": wlayout(np.asarray(Wq, np.float32)[rows, :]),
            "wk": wlayout(np.asarray(Wk, np.float32)[rows, :]),
            "wv": wlayout(np.asarray(Wv, np.float32)[rows, :]),
            "wo": _bf16(np.asarray(Wo, np.float32)[:, rows].T),
            "mask": mask,
        })
    return in_maps


def _run(inputs, trace=False, tmpdir=None):
    from concourse.bass_utils import run_bass_kernel_spmd
    nc = _get_nc()
    in_maps = _prepare_in_maps(inputs["x"], inputs["Wq"], inputs["Wk"],
                               inputs["Wv"], inputs["Wo"])
    res = run_bass_kernel_spmd(nc, in_maps, core_ids=list(range(NCORES)),
                               trace=trace, tmpdir=tmpdir)
    acc = np.zeros((D, NT), np.float32)
    for r in res.results:
        acc += r["out"].astype(np.float32)
    out = acc.T.reshape(B, S, D) + np.asarray(inputs["bo"], np.float32)
    return out.astype(np.float32), res


def kernel(**inputs):
    out, _ = _run(inputs)
    return out


def kernel_traced(tmpdir=None, **inputs):
    out, res = _run(inputs, trace=True, tmpdir=tmpdir)
    return out, res


# revision 19
# speedup vs baseline: 1.1961x; 1.1961x over previous
"""Multi-head causal attention on 8 TRN2 NeuronCores.

B=2, S=2048, D=1024, H=16 heads, head_dim=64. Tensor-parallel over heads:
core c owns heads {2c, 2c+1}. Each core:
  stage 1 (per 512-token piece): qT/kT/vT = W_c @ x.T (feature-major,
           bf16 matmuls, fp32 psum), then v -> token-major via PE
           transpose with a ones column per head appended (gives the
           softmax denominator for free).
  stage 2: attention in scoresT (k-major) layout, joint over the two
           heads: for each 128-wide k-block j, both heads' score
           matmuls (64-deep contractions at PE row groups 0 and 64)
           are issued back-to-back into one shared [128,1024] PSUM
           tile so the hardware runs them concurrently.  One ScalarE
           exp covers both heads; causal handling is exact per block:
           fully-masked q-columns are never computed, and the single
           128-wide triangular window is masked with one small
           multiply (alternating VectorE/GpSimd).  ctxT' accumulation
           = [v|1].T @ pT per head; normalization by the ones-row sum
           via reciprocal_approx_fast + partition broadcast.
  stage 3: partial output projection split per head into 64-deep
           contractions at row groups 0/64 so consecutive f-blocks
           overlap on the PE; woven into the round stream.
A short burst of dummy matmuls at t=0 warms the PE HAM clock gate
during the initial x-DMA wait.  Host sums the 8 partial outputs and
adds the bias.
"""
import numpy as np
import ml_dtypes

B, S, D, H = 2, 2048, 1024, 16
HD = 64          # head dim
NT = B * S       # 4096 tokens
P = 128          # partitions
NCORES = 8
HPC = 2          # heads per core
NM = S // 512    # 4 q-pieces per batch
NP = NT // 512   # 8 token pieces overall
VCB = 2 * (HD + 1)   # 130: v block cols: h0 feats+1, h1 feats+1

_cache = {}


def _build():
    import concourse.bass as bass
    import concourse.mybir as mybir
    from concourse import bacc
    import concourse.tile as tile
    from concourse.masks import make_identity

    BF16 = mybir.dt.bfloat16
    F32 = mybir.dt.float32
    Exp = mybir.ActivationFunctionType.Exp

    nc = bacc.Bacc("TRN2", target_bir_lowering=False, debug=False,
                   num_devices=NCORES)

    xT_d = nc.dram_tensor("xT", [D, NT], BF16, kind="ExternalInput")
    wq_d = nc.dram_tensor("wq", [P, D], BF16, kind="ExternalInput")
    wk_d = nc.dram_tensor("wk", [P, D], BF16, kind="ExternalInput")
    wv_d = nc.dram_tensor("wv", [P, D], BF16, kind="ExternalInput")
    wo_d = nc.dram_tensor("wo", [P, D], BF16, kind="ExternalInput")
    mask_d = nc.dram_tensor("mask", [P, 256], BF16, kind="ExternalInput")
    out_d = nc.dram_tensor("out", [D, NT], BF16, kind="ExternalOutput")

    with tile.TileContext(nc) as tc:
        with tc.tile_pool(name="const", bufs=1) as const, \
             tc.tile_pool(name="xp", bufs=1) as xp, \
             tc.tile_pool(name="qk", bufs=1) as qk, \
             tc.tile_pool(name="misc", bufs=4) as misc, \
             tc.tile_pool(name="stg", bufs=6) as stg, \
             tc.tile_pool(name="pt", bufs=8) as ptp, \
             tc.tile_pool(name="pp", bufs=2, space="PSUM") as pp, \
             tc.tile_pool(name="sc", bufs=2, space="PSUM") as scp, \
             tc.tile_pool(name="cx", bufs=2, space="PSUM") as cxp:

            # ---- constants / weights ----
            # x piece 0 first so stage 1 can begin ASAP; wq/wk/wv next;
            # wo + the mask are not needed until much later
            x_sb = [xp.tile([P, NT], BF16, tag=f"x{c}", name=f"x{c}")
                    for c in range(8)]
            # spread piece-0 chunk DMA issues across four engine queues:
            # a single queue issues one DMA descriptor per ~0.65us, which
            # would serialize the latency-critical first piece
            iss = [nc.sync, nc.sync, nc.sync, nc.scalar,
                   nc.scalar, nc.gpsimd, nc.gpsimd, nc.gpsimd]
            for c in range(8):
                iss[c].dma_start(x_sb[c][:, 0:512],
                                 xT_d.ap()[c * P:(c + 1) * P, 0:512])
            w_sb = {}
            for name, dd in (("wq", wq_d), ("wk", wk_d), ("wv", wv_d)):
                t = const.tile([P, D], BF16, tag=name)
                nc.sync.dma_start(t[:], dd.ap())
                w_sb[name] = t
            for n in range(1, NP):
                cols = slice(n * 512, (n + 1) * 512)
                for c in range(8):
                    nc.sync.dma_start(x_sb[c][:, cols],
                                      xT_d.ap()[c * P:(c + 1) * P, cols])
            wo = const.tile([P, D], BF16, tag="wo")
            nc.sync.dma_start(wo[:], wo_d.ap())
            w_sb["wo"] = wo
            mask_sb = const.tile([P, 256], BF16, tag="mask")
            nc.sync.dma_start(mask_sb[:], mask_d.ap())
            ident = const.tile([P, P], BF16, tag="ident")
            make_identity(nc, ident[:])

            # warm up the PE clock (the HAM throttle holds the PE at
            # half rate until it sees ~3.4us of sustained matmul
            # activity) while the first x DMAs are in flight
            warm = const.tile([P, 256], BF16, tag="warm")
            nc.vector.memset(warm[:], 0.0)
            wps = pp.tile([P, 512], F32, tag="p1", name="warmps")
            for i in range(28):
                nc.tensor.matmul(wps[:, 0:256], warm[:, 0:128], warm[:],
                                 start=True, stop=True)

            qT = qk.tile([P, NT], BF16, tag="qT")
            kT = qk.tile([P, NT], BF16, tag="kT")
            vT = qk.tile([P, NT], BF16, tag="vT")
            v_sb = qk.tile([P, (NT // P) * VCB], BF16, tag="v")
            nc.gpsimd.memset(v_sb[:], 1.0)
            ctxT = qk.tile([P, NT], BF16, tag="ctxT")

            # ---- stage 1 sub-units (half-size so they slot between
            # attention rounds without hogging the PE) ----
            def s1_proj(n, wname, dst):
                # self-contained unit: the psum tile opens and closes in
                # one pump so it never pins a p1 ring slot across rounds
                cols = slice(n * 512, (n + 1) * 512)
                w = w_sb[wname]
                ps = pp.tile([P, 512], F32, tag="p1", name=f"p1_{wname}_{n}")
                for cc in range(8):
                    nc.tensor.matmul(ps[:], w[:, cc * P:(cc + 1) * P],
                                     x_sb[cc][:, cols],
                                     start=(cc == 0), stop=(cc == 7))
                # GpSimd cannot read PSUM; ScalarE is reserved for exp,
                # so all stage-1 casts go to VectorE
                nc.vector.tensor_copy(dst[:, cols], ps[:])

            def s1_vtrans(n, half):
                # v -> token-major for 2 of the 4 blocks of this piece
                for t in range(4 * n + 2 * half, 4 * n + 2 * half + 2):
                    pst = pp.tile([P, P], BF16, tag="p1", name=f"ptr{t}")
                    nc.tensor.transpose(pst[:], vT[:, t * P:(t + 1) * P],
                                        ident[:])
                    # one 3D-AP copy places both heads' 64 feat cols
                    # (strides: head 65, feat 1), skipping the ones cols
                    dst3 = v_sb[:, t * VCB:(t + 1) * VCB].rearrange(
                        "p (h f) -> p h f", f=HD + 1)[:, :, 0:HD]
                    src3 = pst[:, :].rearrange("p (h f) -> p h f", f=HD)
                    nc.vector.tensor_copy(dst3, src3)

            q1 = []
            for n in range(NP):
                for wname, dst in (("wq", qT), ("wk", kT), ("wv", vT)):
                    q1.append((n, lambda n=n, w=wname, d=dst:
                               s1_proj(n, w, d)))
                q1 += [(n, lambda n=n: s1_vtrans(n, 0)),
                       (n, lambda n=n: s1_vtrans(n, 1))]

            i1 = 0
            done1 = -1

            def pump_one():
                # exactly one stage-1 unit: issuing whole pieces between
                # rounds gives them scheduler priority over later
                # attention rounds and serializes the pipeline
                nonlocal i1, done1
                if i1 < len(q1):
                    n, fn = q1[i1]
                    fn()
                    if i1 + 1 >= len(q1) or q1[i1 + 1][0] != n:
                        done1 = n
                    i1 += 1

            def pump_q1(need):
                while done1 < need and i1 < len(q1):
                    pump_one()

            # ---- stage 3: partial output projection; staging casts on
            # VectorE with an occasional ScalarE one (GpSimd cannot
            # read PSUM) ----
            veng = [nc.vector, nc.vector, nc.vector, nc.scalar]
            s3_ctr = [0]

            def s3_quarter(n, qtr):
                cols = slice(n * 512, (n + 1) * 512)
                for f in range(qtr * 2, qtr * 2 + 2):
                    pso = pp.tile([P, 512], F32, tag="p1",
                                  name=f"p3_{f}_{n}")
                    nc.tensor.matmul(pso[:], wo[:, f * P:(f + 1) * P],
                                     ctxT[:, cols], start=True, stop=True)
                    st = stg.tile([P, 512], BF16, tag="st",
                                  name=f"st_{f}_{n}")
                    eng = veng[s3_ctr[0] % 4]
                    if eng is nc.scalar:
                        eng.copy(st[:], pso[:])
                    else:
                        eng.tensor_copy(st[:], pso[:])
                    s3_ctr[0] += 1
                    nc.sync.dma_start(
                        out_d.ap()[f * P:(f + 1) * P, cols], st[:])

            def normalize(b, m, hl, cx):
                hbase = hl * HD
                qc0 = b * S + m * 512
                sm = misc.tile([1, 512], F32, tag="sm",
                               name=f"sm_{b}_{m}_{hl}")
                nc.vector.tensor_copy(sm[:], cx[HD:HD + 1, :])
                rc = misc.tile([1, 512], F32, tag="rc",
                               name=f"rc_{b}_{m}_{hl}")
                nc.vector.reciprocal_approx_fast(rc[:], sm[:])
                bc = misc.tile([HD, 512], F32, tag="bc",
                               name=f"bc_{b}_{m}_{hl}")
                nc.gpsimd.partition_broadcast(bc[:], rc[:])
                nc.vector.tensor_mul(
                    ctxT[hbase:hbase + HD, qc0:qc0 + 512],
                    cx[0:HD, :], bc[:])

            # ---- main loop: per (b, m) group, per k-block rounds ----
            s3q = []
            rnd = 0
            meng = [nc.vector, nc.vector]

            for b in range(B):
                for m in range(NM):
                    pump_q1(b * NM + m)
                    njs = 4 * m + 4
                    qc0 = b * S + m * 512
                    cx = [cxp.tile([HD + 1, 512], F32, tag="cx",
                                   name=f"cx_{b}_{m}_{hl}")
                          for hl in range(HPC)]
                    for j in range(njs):
                        off = 128 * max(0, j - 4 * m)
                        kc0 = b * S + j * P
                        scs = scp.tile([P, 1024], F32, tag="sc",
                                       name=f"sc_{b}_{m}_{j}")
                        # both heads' score matmuls issued back-to-back:
                        # 64-deep contractions at PE row groups 0 / 64
                        # run concurrently
                        for hl in range(HPC):
                            hb = hl * HD
                            nc.tensor.matmul(
                                scs[:, hl * 512 + off:(hl + 1) * 512],
                                kT[hb:hb + HD, kc0:kc0 + P],
                                qT[hb:hb + HD, qc0 + off:qc0 + 512],
                                start=True, stop=True,
                                tile_position=(hb, 0))
                        # one joint exp over both heads' unmasked region
                        pt = ptp.tile([P, 1024], BF16, tag="pt",
                                      name=f"pt_{b}_{m}_{j}")
                        if off:
                            nc.scalar.activation(
                                pt[:].rearrange("p (a c) -> p a c",
                                                a=2)[:, :, off:512],
                                scs[:].rearrange("p (a c) -> p a c",
                                                 a=2)[:, :, off:512],
                                Exp, scale=0.125)
                        else:
                            nc.scalar.activation(pt[:], scs[:], Exp,
                                                 scale=0.125)
                        if j >= 4 * m:
                            # diagonal block: 128-wide triangular window
                            # at [off, off+128) in each half
                            pt3 = pt[:].rearrange(
                                "p (a c) -> p a c", a=2)[:, :, off:off + 128]
                            m3 = mask_sb[:, :].rearrange(
                                "p (a c) -> p a c", a=2)
                            meng[j % 2].tensor_mul(pt3, pt3, m3)
                        for hl in range(HPC):
                            vb = (b * (S // P) + j) * VCB + hl * (HD + 1)
                            nc.tensor.matmul(
                                cx[hl][:, off:512],
                                v_sb[:, vb:vb + HD + 1],
                                pt[:, hl * 512 + off:(hl + 1) * 512],
                                start=(j == 0), stop=(j == njs - 1))
                        # stage-1 / stage-3 filler keeps the PE queue fed
                        pump_one()
                        if s3q:
                            s3_quarter(*s3q.pop(0))
                        if len(s3q) > 3:
                            s3_quarter(*s3q.pop(0))
                        rnd += 1
                    for hl in range(HPC):
                        normalize(b, m, hl, cx[hl])
                    n = b * NM + m
                    s3q += [(n, 0), (n, 1), (n, 2), (n, 3)]
            pump_q1(NP)
            for n, qtr in s3q:
                s3_quarter(n, qtr)
    nc.compile()
    return nc


def _get_nc():
    if "nc" not in _cache:
        _cache["nc"] = _build()
    return _cache["nc"]


def _bf16(a):
    return np.ascontiguousarray(a).astype(ml_dtypes.bfloat16)


def _prepare_in_maps(x, Wq, Wk, Wv, Wo):
    xT = _bf16(np.asarray(x, np.float32).reshape(NT, D).T)
    # triangular window mask (p <= c), duplicated for the two halves
    pp_ = np.arange(P)[:, None]
    cc = np.arange(P)[None, :]
    tri = (pp_ <= cc).astype(np.float32)
    mask = _bf16(np.concatenate([tri, tri], axis=1))

    def wlayout(Wslice):  # [128 feats, 1024 d] -> [p, cc*128+f]
        return _bf16(Wslice.reshape(P, 8, P).transpose(2, 1, 0)
                     .reshape(P, D))

    in_maps = []
    for c in range(NCORES):
        rows = slice(c * P, (c + 1) * P)
        in_maps.append({
            "xT": xT,
            "wq": wlayout(np.asarray(Wq, np.float32)[rows, :]),
            "wk": wlayout(np.asarray(Wk, np.float32)[rows, :]),
            "wv": wlayout(np.asarray(Wv, np.float32)[rows, :]),
            "wo": _bf16(np.asarray(Wo, np.float32)[:, rows].T),
            "mask": mask,
        })
    return in_maps


def _run(inputs, trace=False, tmpdir=None):
    from concourse.bass_utils import run_bass_kernel_spmd
    nc = _get_nc()
    in_maps = _prepare_in_maps(inputs["x"], inputs["Wq"], inputs["Wk"],
                               inputs["Wv"], inputs["Wo"])
    res = run_bass_kernel_spmd(nc, in_maps, core_ids=list(range(NCORES)),
                               trace=trace, tmpdir=tmpdir)
    acc = np.zeros((D, NT), np.float32)
    for r in res.results:
        acc += r["out"].astype(np.float32)
    out = acc.T.reshape(B, S, D) + np.asarray(inputs["bo"], np.float32)
    return out.astype(np.float32), res


def kernel(**inputs):
    out, _ = _run(inputs)
    return out


def kernel_traced(tmpdir=None, **inputs):
    out, res = _run(inputs, trace=True, tmpdir=tmpdir)
    return out, res


# revision 20
# speedup vs baseline: 1.4025x; 1.1725x over previous
"""Multi-head causal attention on 8 TRN2 NeuronCores.

B=2, S=2048, D=1024, H=16 heads, head_dim=64. Tensor-parallel over heads:
core c owns heads {2c, 2c+1}. Each core:
  stage 1 (per 512-token piece): qT/kT/vT = W_c @ x.T (feature-major,
           bf16 matmuls, fp32 psum), then v -> token-major via PE
           transpose with a ones column per head appended (gives the
           softmax denominator for free).
  stage 2: attention in scoresT (k-major) layout, joint over the two
           heads: for each 128-wide k-block j, both heads' score
           matmuls (64-deep contractions at PE row groups 0 and 64)
           are issued back-to-back into one shared [128,1024] PSUM
           tile so the hardware runs them concurrently.  One ScalarE
           exp covers both heads; causal handling is exact per block:
           fully-masked q-columns are never computed, and the single
           128-wide triangular window is masked with one small
           multiply (alternating VectorE/GpSimd).  ctxT' accumulation
           = [v|1].T @ pT per head; normalization by the ones-row sum
           via reciprocal_approx_fast + partition broadcast.
  stage 3: partial output projection split per head into 64-deep
           contractions at row groups 0/64 so consecutive f-blocks
           overlap on the PE; woven into the round stream.
A short burst of dummy matmuls at t=0 warms the PE HAM clock gate
during the initial x-DMA wait.  Host sums the 8 partial outputs and
adds the bias.
"""
import numpy as np
import ml_dtypes

B, S, D, H = 2, 2048, 1024, 16
HD = 64          # head dim
NT = B * S       # 4096 tokens
P = 128          # partitions
NCORES = 8
HPC = 2          # heads per core
NM = S // 512    # 4 q-pieces per batch
NP = NT // 512   # 8 token pieces overall
VCB = 2 * (HD + 1)   # 130: v block cols: h0 feats+1, h1 feats+1

_cache = {}


def _build():
    import concourse.bass as bass
    import concourse.mybir as mybir
    from concourse import bacc
    import concourse.tile as tile
    from concourse.masks import make_identity

    BF16 = mybir.dt.bfloat16
    F32 = mybir.dt.float32
    Exp = mybir.ActivationFunctionType.Exp

    nc = bacc.Bacc("TRN2", target_bir_lowering=False, debug=False,
                   num_devices=NCORES)

    xT_d = nc.dram_tensor("xT", [D, NT], BF16, kind="ExternalInput")
    wq_d = nc.dram_tensor("wq", [P, D], BF16, kind="ExternalInput")
    wk_d = nc.dram_tensor("wk", [P, D], BF16, kind="ExternalInput")
    wv_d = nc.dram_tensor("wv", [P, D], BF16, kind="ExternalInput")
    wo_d = nc.dram_tensor("wo", [P, D], BF16, kind="ExternalInput")
    mask_d = nc.dram_tensor("mask", [P, 256], BF16, kind="ExternalInput")
    out_d = nc.dram_tensor("out", [D, NT], BF16, kind="ExternalOutput")

    with tile.TileContext(nc) as tc:
        with tc.tile_pool(name="const", bufs=1) as const, \
             tc.tile_pool(name="xp", bufs=1) as xp, \
             tc.tile_pool(name="qk", bufs=1) as qk, \
             tc.tile_pool(name="misc", bufs=4) as misc, \
             tc.tile_pool(name="stg", bufs=6) as stg, \
             tc.tile_pool(name="pt", bufs=8) as ptp, \
             tc.tile_pool(name="pp", bufs=2, space="PSUM") as pp, \
             tc.tile_pool(name="sc", bufs=2, space="PSUM") as scp, \
             tc.tile_pool(name="cx", bufs=2, space="PSUM") as cxp:

            # ---- constants / weights ----
            # x piece 0 first so stage 1 can begin ASAP; wq/wk/wv next;
            # wo + the mask are not needed until much later
            x_sb = [xp.tile([P, NT], BF16, tag=f"x{c}", name=f"x{c}")
                    for c in range(8)]
            # spread piece-0 chunk DMA issues across four engine queues:
            # a single queue issues one DMA descriptor per ~0.65us, which
            # would serialize the latency-critical first piece
            iss = [nc.sync, nc.sync, nc.sync, nc.scalar,
                   nc.scalar, nc.gpsimd, nc.gpsimd, nc.gpsimd]
            for c in range(8):
                iss[c].dma_start(x_sb[c][:, 0:512],
                                 xT_d.ap()[c * P:(c + 1) * P, 0:512])
            w_sb = {}
            for name, dd in (("wq", wq_d), ("wk", wk_d), ("wv", wv_d)):
                t = const.tile([P, D], BF16, tag=name)
                nc.sync.dma_start(t[:], dd.ap())
                w_sb[name] = t
            for n in range(1, NP):
                cols = slice(n * 512, (n + 1) * 512)
                for c in range(8):
                    nc.sync.dma_start(x_sb[c][:, cols],
                                      xT_d.ap()[c * P:(c + 1) * P, cols])
            wo = const.tile([P, D], BF16, tag="wo")
            nc.sync.dma_start(wo[:], wo_d.ap())
            w_sb["wo"] = wo
            mask_sb = const.tile([P, 256], BF16, tag="mask")
            nc.sync.dma_start(mask_sb[:], mask_d.ap())
            ident = const.tile([P, P], BF16, tag="ident")
            make_identity(nc, ident[:])

            # warm up the PE clock (the HAM throttle holds the PE at
            # half rate until it sees ~3.4us of sustained matmul
            # activity) while the first x DMAs are in flight
            warm = const.tile([P, 256], BF16, tag="warm")
            nc.vector.memset(warm[:], 0.0)
            wps = pp.tile([P, 512], F32, tag="p1", name="warmps")
            for i in range(28):
                nc.tensor.matmul(wps[:, 0:256], warm[:, 0:128], warm[:],
                                 start=True, stop=True)

            qT = qk.tile([P, NT], BF16, tag="qT")
            kT = qk.tile([P, NT], BF16, tag="kT")
            vT = qk.tile([P, NT], BF16, tag="vT")
            v_sb = qk.tile([P, (NT // P) * VCB], BF16, tag="v")
            nc.gpsimd.memset(v_sb[:], 1.0)
            ctxT = qk.tile([P, NT], BF16, tag="ctxT")

            # ---- stage 1 sub-units (half-size so they slot between
            # attention rounds without hogging the PE) ----
            def s1_proj(n, wname, dst):
                # self-contained unit: the psum tile opens and closes in
                # one pump so it never pins a p1 ring slot across rounds
                cols = slice(n * 512, (n + 1) * 512)
                w = w_sb[wname]
                ps = pp.tile([P, 512], F32, tag="p1", name=f"p1_{wname}_{n}")
                for cc in range(8):
                    nc.tensor.matmul(ps[:], w[:, cc * P:(cc + 1) * P],
                                     x_sb[cc][:, cols],
                                     start=(cc == 0), stop=(cc == 7))
                # GpSimd cannot read PSUM; ScalarE is reserved for exp,
                # so all stage-1 casts go to VectorE
                nc.vector.tensor_copy(dst[:, cols], ps[:])

            def s1_vtrans(n, half):
                # v -> token-major for 2 of the 4 blocks of this piece
                for t in range(4 * n + 2 * half, 4 * n + 2 * half + 2):
                    pst = pp.tile([P, P], BF16, tag="p1", name=f"ptr{t}")
                    nc.tensor.transpose(pst[:], vT[:, t * P:(t + 1) * P],
                                        ident[:])
                    # one 3D-AP copy places both heads' 64 feat cols
                    # (strides: head 65, feat 1), skipping the ones cols
                    dst3 = v_sb[:, t * VCB:(t + 1) * VCB].rearrange(
                        "p (h f) -> p h f", f=HD + 1)[:, :, 0:HD]
                    src3 = pst[:, :].rearrange("p (h f) -> p h f", f=HD)
                    nc.vector.tensor_copy(dst3, src3)

            q1 = []
            for n in range(NP):
                for wname, dst in (("wq", qT), ("wk", kT), ("wv", vT)):
                    q1.append((n, lambda n=n, w=wname, d=dst:
                               s1_proj(n, w, d)))
                q1 += [(n, lambda n=n: s1_vtrans(n, 0)),
                       (n, lambda n=n: s1_vtrans(n, 1))]

            i1 = 0
            done1 = -1

            def pump_one():
                # exactly one stage-1 unit: issuing whole pieces between
                # rounds gives them scheduler priority over later
                # attention rounds and serializes the pipeline.  The
                # tile_wait_until hint tells the scheduler when this
                # piece's x DMA actually lands (~10GB/s per queue, ~9
                # queues, ~6.5us per piece) so the frozen engine-queue
                # order doesn't put stage-1 work for late pieces ahead
                # of attention ops that are ready sooner.
                nonlocal i1, done1
                if i1 < len(q1):
                    n, fn = q1[i1]
                    with tc.tile_wait_until(0.009 + 0.0065 * n):
                        fn()
                    if i1 + 1 >= len(q1) or q1[i1 + 1][0] != n:
                        done1 = n
                    i1 += 1

            def pump_q1(need):
                while done1 < need and i1 < len(q1):
                    pump_one()

            # ---- stage 3: partial output projection; staging casts on
            # VectorE with an occasional ScalarE one (GpSimd cannot
            # read PSUM) ----
            veng = [nc.vector, nc.vector, nc.vector, nc.scalar]
            s3_ctr = [0]

            def s3_quarter(n, qtr):
                cols = slice(n * 512, (n + 1) * 512)
                for f in range(qtr * 2, qtr * 2 + 2):
                    pso = pp.tile([P, 512], F32, tag="p1",
                                  name=f"p3_{f}_{n}")
                    nc.tensor.matmul(pso[:], wo[:, f * P:(f + 1) * P],
                                     ctxT[:, cols], start=True, stop=True)
                    st = stg.tile([P, 512], BF16, tag="st",
                                  name=f"st_{f}_{n}")
                    eng = veng[s3_ctr[0] % 4]
                    if eng is nc.scalar:
                        eng.copy(st[:], pso[:])
                    else:
                        eng.tensor_copy(st[:], pso[:])
                    s3_ctr[0] += 1
                    nc.sync.dma_start(
                        out_d.ap()[f * P:(f + 1) * P, cols], st[:])

            def normalize(b, m, hl, cx):
                hbase = hl * HD
                qc0 = b * S + m * 512
                sm = misc.tile([1, 512], F32, tag="sm",
                               name=f"sm_{b}_{m}_{hl}")
                nc.vector.tensor_copy(sm[:], cx[HD:HD + 1, :])
                rc = misc.tile([1, 512], F32, tag="rc",
                               name=f"rc_{b}_{m}_{hl}")
                nc.vector.reciprocal_approx_fast(rc[:], sm[:])
                bc = misc.tile([HD, 512], F32, tag="bc",
                               name=f"bc_{b}_{m}_{hl}")
                nc.gpsimd.partition_broadcast(bc[:], rc[:])
                nc.vector.tensor_mul(
                    ctxT[hbase:hbase + HD, qc0:qc0 + 512],
                    cx[0:HD, :], bc[:])

            # ---- main loop: per (b, m) group, per k-block rounds ----
            s3q = []
            rnd = 0
            meng = [nc.vector, nc.vector]

            for b in range(B):
                for m in range(NM):
                    pump_q1(b * NM + m)
                    njs = 4 * m + 4
                    qc0 = b * S + m * 512
                    cx = [cxp.tile([HD + 1, 512], F32, tag="cx",
                                   name=f"cx_{b}_{m}_{hl}")
                          for hl in range(HPC)]
                    for j in range(njs):
                        off = 128 * max(0, j - 4 * m)
                        kc0 = b * S + j * P
                        scs = scp.tile([P, 1024], F32, tag="sc",
                                       name=f"sc_{b}_{m}_{j}")
                        # both heads' score matmuls issued back-to-back:
                        # 64-deep contractions at PE row groups 0 / 64
                        # run concurrently
                        for hl in range(HPC):
                            hb = hl * HD
                            nc.tensor.matmul(
                                scs[:, hl * 512 + off:(hl + 1) * 512],
                                kT[hb:hb + HD, kc0:kc0 + P],
                                qT[hb:hb + HD, qc0 + off:qc0 + 512],
                                start=True, stop=True,
                                tile_position=(hb, 0))
                        # one joint exp over both heads' unmasked region
                        pt = ptp.tile([P, 1024], BF16, tag="pt",
                                      name=f"pt_{b}_{m}_{j}")
                        if off:
                            nc.scalar.activation(
                                pt[:].rearrange("p (a c) -> p a c",
                                                a=2)[:, :, off:512],
                                scs[:].rearrange("p (a c) -> p a c",
                                                 a=2)[:, :, off:512],
                                Exp, scale=0.125)
                        else:
                            nc.scalar.activation(pt[:], scs[:], Exp,
                                                 scale=0.125)
                        if j >= 4 * m:
                            # diagonal block: 128-wide triangular window
                            # at [off, off+128) in each half
                            pt3 = pt[:].rearrange(
                                "p (a c) -> p a c", a=2)[:, :, off:off + 128]
                            m3 = mask_sb[:, :].rearrange(
                                "p (a c) -> p a c", a=2)
                            meng[j % 2].tensor_mul(pt3, pt3, m3)
                        for hl in range(HPC):
                            vb = (b * (S // P) + j) * VCB + hl * (HD + 1)
                            nc.tensor.matmul(
                                cx[hl][:, off:512],
                                v_sb[:, vb:vb + HD + 1],
                                pt[:, hl * 512 + off:(hl + 1) * 512],
                                start=(j == 0), stop=(j == njs - 1))
                        # stage-1 / stage-3 filler keeps the PE queue fed
                        pump_one()
                        if s3q:
                            s3_quarter(*s3q.pop(0))
                        if len(s3q) > 3:
                            s3_quarter(*s3q.pop(0))
                        rnd += 1
                    for hl in range(HPC):
                        normalize(b, m, hl, cx[hl])
                    n = b * NM + m
                    s3q += [(n, 0), (n, 1), (n, 2), (n, 3)]
            pump_q1(NP)
            for n, qtr in s3q:
                s3_quarter(n, qtr)
    nc.compile()
    return nc


def _get_nc():
    if "nc" not in _cache:
        _cache["nc"] = _build()
    return _cache["nc"]


def _bf16(a):
    return np.ascontiguousarray(a).astype(ml_dtypes.bfloat16)


def _prepare_in_maps(x, Wq, Wk, Wv, Wo):
    xT = _bf16(np.asarray(x, np.float32).reshape(NT, D).T)
    # triangular window mask (p <= c), duplicated for the two halves
    pp_ = np.arange(P)[:, None]
    cc = np.arange(P)[None, :]
    tri = (pp_ <= cc).astype(np.float32)
    mask = _bf16(np.concatenate([tri, tri], axis=1))

    def wlayout(Wslice):  # [128 feats, 1024 d] -> [p, cc*128+f]
        return _bf16(Wslice.reshape(P, 8, P).transpose(2, 1, 0)
                     .reshape(P, D))

    in_maps = []
    for c in range(NCORES):
        rows = slice(c * P, (c + 1) * P)
        in_maps.append({
            "xT": xT,
            "wq": wlayout(np.asarray(Wq, np.float32)[rows, :]),
            "wk": wlayout(np.asarray(Wk, np.float32)[rows, :]),
            "wv": wlayout(np.asarray(Wv, np.float32)[rows, :]),
            "wo": _bf16(np.asarray(Wo, np.float32)[:, rows].T),
            "mask": mask,
        })
    return in_maps


def _run(inputs, trace=False, tmpdir=None):
    from concourse.bass_utils import run_bass_kernel_spmd
    nc = _get_nc()
    in_maps = _prepare_in_maps(inputs["x"], inputs["Wq"], inputs["Wk"],
                               inputs["Wv"], inputs["Wo"])
    res = run_bass_kernel_spmd(nc, in_maps, core_ids=list(range(NCORES)),
                               trace=trace, tmpdir=tmpdir)
    acc = np.zeros((D, NT), np.float32)
    for r in res.results:
        acc += r["out"].astype(np.float32)
    out = acc.T.reshape(B, S, D) + np.asarray(inputs["bo"], np.float32)
    return out.astype(np.float32), res


def kernel(**inputs):
    out, _ = _run(inputs)
    return out


def kernel_traced(tmpdir=None, **inputs):
    out, res = _run(inputs, trace=True, tmpdir=tmpdir)
    return out, res


# revision 21
# speedup vs baseline: 1.4928x; 1.0644x over previous
"""Multi-head causal attention on 8 TRN2 NeuronCores.

B=2, S=2048, D=1024, H=16 heads, head_dim=64. Tensor-parallel over heads:
core c owns heads {2c, 2c+1}. Each core:
  stage 1 (per 512-token piece): qT/kT/vT = W_c @ x.T (feature-major,
           bf16 matmuls, fp32 psum), then v -> token-major via PE
           transpose with a ones column per head appended (gives the
           softmax denominator for free).
  stage 2: attention in scoresT (k-major) layout, joint over the two
           heads: for each 128-wide k-block j, both heads' score
           matmuls (64-deep contractions at PE row groups 0 and 64)
           are issued back-to-back into one shared [128,1024] PSUM
           tile so the hardware runs them concurrently.  One ScalarE
           exp covers both heads; causal handling is exact per block:
           fully-masked q-columns are never computed, and the single
           128-wide triangular window is masked with one small
           multiply (alternating VectorE/GpSimd).  ctxT' accumulation
           = [v|1].T @ pT per head; normalization by the ones-row sum
           via reciprocal_approx_fast + partition broadcast.
  stage 3: partial output projection split per head into 64-deep
           contractions at row groups 0/64 so consecutive f-blocks
           overlap on the PE; woven into the round stream.
A short burst of dummy matmuls at t=0 warms the PE HAM clock gate
during the initial x-DMA wait.  Host sums the 8 partial outputs and
adds the bias.
"""
import numpy as np
import ml_dtypes

B, S, D, H = 2, 2048, 1024, 16
HD = 64          # head dim
NT = B * S       # 4096 tokens
P = 128          # partitions
NCORES = 8
HPC = 2          # heads per core
NM = S // 512    # 4 q-pieces per batch
NP = NT // 512   # 8 token pieces overall
VCB = 2 * (HD + 1)   # 130: v block cols: h0 feats+1, h1 feats+1

_cache = {}


def _build():
    import concourse.bass as bass
    import concourse.mybir as mybir
    from concourse import bacc
    import concourse.tile as tile
    from concourse.masks import make_identity

    BF16 = mybir.dt.bfloat16
    F32 = mybir.dt.float32
    Exp = mybir.ActivationFunctionType.Exp

    nc = bacc.Bacc("TRN2", target_bir_lowering=False, debug=False,
                   num_devices=NCORES)

    xT_d = nc.dram_tensor("xT", [D, NT], BF16, kind="ExternalInput")
    wq_d = nc.dram_tensor("wq", [P, D], BF16, kind="ExternalInput")
    wk_d = nc.dram_tensor("wk", [P, D], BF16, kind="ExternalInput")
    wv_d = nc.dram_tensor("wv", [P, D], BF16, kind="ExternalInput")
    wo_d = nc.dram_tensor("wo", [P, D], BF16, kind="ExternalInput")
    mask_d = nc.dram_tensor("mask", [P, 256], BF16, kind="ExternalInput")
    out_d = nc.dram_tensor("out", [D, NT], BF16, kind="ExternalOutput")

    with tile.TileContext(nc) as tc:
        with tc.tile_pool(name="const", bufs=1) as const, \
             tc.tile_pool(name="xp", bufs=1) as xp, \
             tc.tile_pool(name="qk", bufs=1) as qk, \
             tc.tile_pool(name="misc", bufs=4) as misc, \
             tc.tile_pool(name="stg", bufs=6) as stg, \
             tc.tile_pool(name="pt", bufs=8) as ptp, \
             tc.tile_pool(name="pp", bufs=2, space="PSUM") as pp, \
             tc.tile_pool(name="sc", bufs=2, space="PSUM") as scp, \
             tc.tile_pool(name="cx", bufs=2, space="PSUM") as cxp:

            # ---- constants / weights ----
            # x piece 0 first so stage 1 can begin ASAP; wq/wk/wv next;
            # wo + the mask are not needed until much later
            x_sb = [xp.tile([P, NT], BF16, tag=f"x{c}", name=f"x{c}")
                    for c in range(8)]
            # spread piece-0 chunk DMA issues across four engine queues:
            # a single queue issues one DMA descriptor per ~0.65us, which
            # would serialize the latency-critical first piece
            iss = [nc.sync, nc.sync, nc.sync, nc.scalar,
                   nc.scalar, nc.gpsimd, nc.gpsimd, nc.gpsimd]
            for c in range(8):
                iss[c].dma_start(x_sb[c][:, 0:512],
                                 xT_d.ap()[c * P:(c + 1) * P, 0:512])
            # mask + weights must land before the first attention group:
            # anything issued after the hoisted x DMAs queues behind
            # ~8 x 12.5us of per-queue transfers
            mask_sb = const.tile([P, 256], BF16, tag="mask")
            nc.sync.dma_start(mask_sb[:], mask_d.ap())
            w_sb = {}
            for name, dd in (("wq", wq_d), ("wk", wk_d), ("wv", wv_d)):
                t = const.tile([P, D], BF16, tag=name)
                nc.sync.dma_start(t[:], dd.ap())
                w_sb[name] = t
            wo = const.tile([P, D], BF16, tag="wo")
            nc.sync.dma_start(wo[:], wo_d.ap())
            w_sb["wo"] = wo
            for n in range(1, NP):
                cols = slice(n * 512, (n + 1) * 512)
                for c in range(8):
                    nc.sync.dma_start(x_sb[c][:, cols],
                                      xT_d.ap()[c * P:(c + 1) * P, cols])
            ident = const.tile([P, P], BF16, tag="ident")
            make_identity(nc, ident[:])

            # warm up the PE clock (the HAM throttle holds the PE at
            # half rate until it sees ~3.4us of sustained matmul
            # activity) while the first x DMAs are in flight
            warm = const.tile([P, 256], BF16, tag="warm")
            nc.vector.memset(warm[:], 0.0)
            wps = pp.tile([P, 512], F32, tag="p1", name="warmps")
            for i in range(28):
                nc.tensor.matmul(wps[:, 0:256], warm[:, 0:128], warm[:],
                                 start=True, stop=True)

            qT = qk.tile([P, NT], BF16, tag="qT")
            kT = qk.tile([P, NT], BF16, tag="kT")
            vT = qk.tile([P, NT], BF16, tag="vT")
            v_sb = qk.tile([P, (NT // P) * VCB], BF16, tag="v")
            nc.gpsimd.memset(v_sb[:], 1.0)
            ctxT = qk.tile([P, NT], BF16, tag="ctxT")

            # ---- stage 1 sub-units (half-size so they slot between
            # attention rounds without hogging the PE) ----
            def s1_proj(n, wname, dst):
                # self-contained unit: the psum tile opens and closes in
                # one pump so it never pins a p1 ring slot across rounds
                cols = slice(n * 512, (n + 1) * 512)
                w = w_sb[wname]
                ps = pp.tile([P, 512], F32, tag="p1", name=f"p1_{wname}_{n}")
                for cc in range(8):
                    nc.tensor.matmul(ps[:], w[:, cc * P:(cc + 1) * P],
                                     x_sb[cc][:, cols],
                                     start=(cc == 0), stop=(cc == 7))
                # GpSimd cannot read PSUM; ScalarE is reserved for exp,
                # so all stage-1 casts go to VectorE
                nc.vector.tensor_copy(dst[:, cols], ps[:])

            def s1_vtrans(n, half):
                # v -> token-major for 2 of the 4 blocks of this piece
                for t in range(4 * n + 2 * half, 4 * n + 2 * half + 2):
                    pst = pp.tile([P, P], BF16, tag="p1", name=f"ptr{t}")
                    nc.tensor.transpose(pst[:], vT[:, t * P:(t + 1) * P],
                                        ident[:])
                    # one 3D-AP copy places both heads' 64 feat cols
                    # (strides: head 65, feat 1), skipping the ones cols
                    dst3 = v_sb[:, t * VCB:(t + 1) * VCB].rearrange(
                        "p (h f) -> p h f", f=HD + 1)[:, :, 0:HD]
                    src3 = pst[:, :].rearrange("p (h f) -> p h f", f=HD)
                    nc.vector.tensor_copy(dst3, src3)

            q1 = []
            for n in range(NP):
                for wname, dst in (("wq", qT), ("wk", kT), ("wv", vT)):
                    q1.append((n, lambda n=n, w=wname, d=dst:
                               s1_proj(n, w, d)))
                q1 += [(n, lambda n=n: s1_vtrans(n, 0)),
                       (n, lambda n=n: s1_vtrans(n, 1))]

            i1 = 0
            done1 = -1

            def pump_one():
                # exactly one stage-1 unit: issuing whole pieces between
                # rounds gives them scheduler priority over later
                # attention rounds and serializes the pipeline.  The
                # tile_wait_until hint tells the scheduler when this
                # piece's x DMA actually lands (~10GB/s per queue, ~9
                # queues, ~6.5us per piece) so the frozen engine-queue
                # order doesn't put stage-1 work for late pieces ahead
                # of attention ops that are ready sooner.
                nonlocal i1, done1
                if i1 < len(q1):
                    n, fn = q1[i1]
                    with tc.tile_wait_until(0.009 + 0.0065 * n):
                        fn()
                    if i1 + 1 >= len(q1) or q1[i1 + 1][0] != n:
                        done1 = n
                    i1 += 1

            def pump_q1(need):
                while done1 < need and i1 < len(q1):
                    pump_one()

            # ---- stage 3: partial output projection; staging casts on
            # VectorE with an occasional ScalarE one (GpSimd cannot
            # read PSUM) ----
            veng = [nc.vector, nc.vector, nc.vector, nc.scalar]
            s3_ctr = [0]

            def s3_quarter(n, qtr):
                cols = slice(n * 512, (n + 1) * 512)
                for f in range(qtr * 2, qtr * 2 + 2):
                    pso = pp.tile([P, 512], F32, tag="p1",
                                  name=f"p3_{f}_{n}")
                    nc.tensor.matmul(pso[:], wo[:, f * P:(f + 1) * P],
                                     ctxT[:, cols], start=True, stop=True)
                    st = stg.tile([P, 512], BF16, tag="st",
                                  name=f"st_{f}_{n}")
                    eng = veng[s3_ctr[0] % 4]
                    if eng is nc.scalar:
                        eng.copy(st[:], pso[:])
                    else:
                        eng.tensor_copy(st[:], pso[:])
                    s3_ctr[0] += 1
                    nc.sync.dma_start(
                        out_d.ap()[f * P:(f + 1) * P, cols], st[:])

            def normalize(b, m, hl, cx):
                hbase = hl * HD
                qc0 = b * S + m * 512
                sm = misc.tile([1, 512], F32, tag="sm",
                               name=f"sm_{b}_{m}_{hl}")
                nc.vector.tensor_copy(sm[:], cx[HD:HD + 1, :])
                rc = misc.tile([1, 512], F32, tag="rc",
                               name=f"rc_{b}_{m}_{hl}")
                nc.vector.reciprocal_approx_fast(rc[:], sm[:])
                bc = misc.tile([HD, 512], F32, tag="bc",
                               name=f"bc_{b}_{m}_{hl}")
                nc.gpsimd.partition_broadcast(bc[:], rc[:])
                nc.vector.tensor_mul(
                    ctxT[hbase:hbase + HD, qc0:qc0 + 512],
                    cx[0:HD, :], bc[:])

            # ---- main loop: per (b, m) group, per k-block rounds ----
            s3q = []
            rnd = 0
            meng = [nc.vector, nc.vector]

            for b in range(B):
                for m in range(NM):
                    pump_q1(b * NM + m)
                    njs = 4 * m + 4
                    qc0 = b * S + m * 512
                    cx = [cxp.tile([HD + 1, 512], F32, tag="cx",
                                   name=f"cx_{b}_{m}_{hl}")
                          for hl in range(HPC)]
                    for j in range(njs):
                        off = 128 * max(0, j - 4 * m)
                        kc0 = b * S + j * P
                        scs = scp.tile([P, 1024], F32, tag="sc",
                                       name=f"sc_{b}_{m}_{j}")
                        # both heads' score matmuls issued back-to-back:
                        # 64-deep contractions at PE row groups 0 / 64
                        # run concurrently
                        for hl in range(HPC):
                            hb = hl * HD
                            nc.tensor.matmul(
                                scs[:, hl * 512 + off:(hl + 1) * 512],
                                kT[hb:hb + HD, kc0:kc0 + P],
                                qT[hb:hb + HD, qc0 + off:qc0 + 512],
                                start=True, stop=True,
                                tile_position=(hb, 0))
                        # one joint exp over both heads' unmasked region
                        pt = ptp.tile([P, 1024], BF16, tag="pt",
                                      name=f"pt_{b}_{m}_{j}")
                        if off:
                            nc.scalar.activation(
                                pt[:].rearrange("p (a c) -> p a c",
                                                a=2)[:, :, off:512],
                                scs[:].rearrange("p (a c) -> p a c",
                                                 a=2)[:, :, off:512],
                                Exp, scale=0.125)
                        else:
                            nc.scalar.activation(pt[:], scs[:], Exp,
                                                 scale=0.125)
                        if j >= 4 * m:
                            # diagonal block: 128-wide triangular window
                            # at [off, off+128) in each half
                            pt3 = pt[:].rearrange(
                                "p (a c) -> p a c", a=2)[:, :, off:off + 128]
                            m3 = mask_sb[:, :].rearrange(
                                "p (a c) -> p a c", a=2)
                            meng[j % 2].tensor_mul(pt3, pt3, m3)
                        for hl in range(HPC):
                            vb = (b * (S // P) + j) * VCB + hl * (HD + 1)
                            nc.tensor.matmul(
                                cx[hl][:, off:512],
                                v_sb[:, vb:vb + HD + 1],
                                pt[:, hl * 512 + off:(hl + 1) * 512],
                                start=(j == 0), stop=(j == njs - 1))
                        # stage-1 / stage-3 filler keeps the PE queue fed
                        pump_one()
                        if s3q:
                            s3_quarter(*s3q.pop(0))
                        if len(s3q) > 3:
                            s3_quarter(*s3q.pop(0))
                        rnd += 1
                    for hl in range(HPC):
                        normalize(b, m, hl, cx[hl])
                    n = b * NM + m
                    s3q += [(n, 0), (n, 1), (n, 2), (n, 3)]
            pump_q1(NP)
            for n, qtr in s3q:
                s3_quarter(n, qtr)
    nc.compile()
    return nc


def _get_nc():
    if "nc" not in _cache:
        _cache["nc"] = _build()
    return _cache["nc"]


def _bf16(a):
    return np.ascontiguousarray(a).astype(ml_dtypes.bfloat16)


def _prepare_in_maps(x, Wq, Wk, Wv, Wo):
    xT = _bf16(np.asarray(x, np.float32).reshape(NT, D).T)
    # triangular window mask (p <= c), duplicated for the two halves
    pp_ = np.arange(P)[:, None]
    cc = np.arange(P)[None, :]
    tri = (pp_ <= cc).astype(np.float32)
    mask = _bf16(np.concatenate([tri, tri], axis=1))

    def wlayout(Wslice):  # [128 feats, 1024 d] -> [p, cc*128+f]
        return _bf16(Wslice.reshape(P, 8, P).transpose(2, 1, 0)
                     .reshape(P, D))

    in_maps = []
    for c in range(NCORES):
        rows = slice(c * P, (c + 1) * P)
        in_maps.append({
            "xT": xT,
            "wq": wlayout(np.asarray(Wq, np.float32)[rows, :]),
            "wk": wlayout(np.asarray(Wk, np.float32)[rows, :]),
            "wv": wlayout(np.asarray(Wv, np.float32)[rows, :]),
            "wo": _bf16(np.asarray(Wo, np.float32)[:, rows].T),
            "mask": mask,
        })
    return in_maps


def _run(inputs, trace=False, tmpdir=None):
    from concourse.bass_utils import run_bass_kernel_spmd
    nc = _get_nc()
    in_maps = _prepare_in_maps(inputs["x"], inputs["Wq"], inputs["Wk"],
                               inputs["Wv"], inputs["Wo"])
    res = run_bass_kernel_spmd(nc, in_maps, core_ids=list(range(NCORES)),
                               trace=trace, tmpdir=tmpdir)
    acc = np.zeros((D, NT), np.float32)
    for r in res.results:
        acc += r["out"].astype(np.float32)
    out = acc.T.reshape(B, S, D) + np.asarray(inputs["bo"], np.float32)
    return out.astype(np.float32), res


def kernel(**inputs):
    out, _ = _run(inputs)
    return out


def kernel_traced(tmpdir=None, **inputs):
    out, res = _run(inputs, trace=True, tmpdir=tmpdir)
    return out, res


# revision 25
# speedup vs baseline: 1.5848x; 1.0616x over previous
"""Multi-head causal attention on 8 TRN2 NeuronCores.

B=2, S=2048, D=1024, H=16 heads, head_dim=64. Tensor-parallel over heads:
core c owns heads {2c, 2c+1}. Each core:
  stage 1 (per 512-token piece): qT/kT/vT = W_c @ x.T (feature-major,
           bf16 matmuls, fp32 psum), then v -> token-major via PE
           transpose with a ones column per head appended (gives the
           softmax denominator for free).
  stage 2: attention in scoresT (k-major) layout, joint over the two
           heads: for each 128-wide k-block j, both heads' score
           matmuls (64-deep contractions at PE row groups 0 and 64)
           are issued back-to-back into one shared [128,1024] PSUM
           tile so the hardware runs them concurrently.  One ScalarE
           exp covers both heads; causal handling is exact per block:
           fully-masked q-columns are never computed, and the single
           128-wide triangular window is masked with one small
           multiply (alternating VectorE/GpSimd).  ctxT' accumulation
           = [v|1].T @ pT per head; normalization by the ones-row sum
           via reciprocal_approx_fast + partition broadcast.
  stage 3: partial output projection split per head into 64-deep
           contractions at row groups 0/64 so consecutive f-blocks
           overlap on the PE; woven into the round stream.
A short burst of dummy matmuls at t=0 warms the PE HAM clock gate
during the initial x-DMA wait.  Host sums the 8 partial outputs and
adds the bias.
"""
import numpy as np
import ml_dtypes

B, S, D, H = 2, 2048, 1024, 16
HD = 64          # head dim
NT = B * S       # 4096 tokens
P = 128          # partitions
NCORES = 8
HPC = 2          # heads per core
NM = S // 512    # 4 q-pieces per batch
NP = NT // 512   # 8 token pieces overall
VCB = 2 * (HD + 1)   # 130: v block cols: h0 feats+1, h1 feats+1

_cache = {}


def _build():
    import concourse.bass as bass
    import concourse.mybir as mybir
    from concourse import bacc
    import concourse.tile as tile
    from concourse.masks import make_identity

    BF16 = mybir.dt.bfloat16
    F32 = mybir.dt.float32
    Exp = mybir.ActivationFunctionType.Exp

    nc = bacc.Bacc("TRN2", target_bir_lowering=False, debug=False,
                   num_devices=NCORES)

    xT_d = nc.dram_tensor("xT", [D, NT], BF16, kind="ExternalInput")
    wq_d = nc.dram_tensor("wq", [P, D], BF16, kind="ExternalInput")
    wk_d = nc.dram_tensor("wk", [P, D], BF16, kind="ExternalInput")
    wv_d = nc.dram_tensor("wv", [P, D], BF16, kind="ExternalInput")
    wo_d = nc.dram_tensor("wo", [P, D], BF16, kind="ExternalInput")
    mask_d = nc.dram_tensor("mask", [P, 256], BF16, kind="ExternalInput")
    out_d = nc.dram_tensor("out", [D, NT], BF16, kind="ExternalOutput")

    with tile.TileContext(nc) as tc:
        with tc.tile_pool(name="const", bufs=1) as const, \
             tc.tile_pool(name="xp", bufs=1) as xp, \
             tc.tile_pool(name="qk", bufs=1) as qk, \
             tc.tile_pool(name="misc", bufs=4) as misc, \
             tc.tile_pool(name="stg", bufs=6) as stg, \
             tc.tile_pool(name="pt", bufs=8) as ptp, \
             tc.tile_pool(name="pp", bufs=2, space="PSUM") as pp, \
             tc.tile_pool(name="sc", bufs=2, space="PSUM") as scp, \
             tc.tile_pool(name="cx", bufs=2, space="PSUM") as cxp:

            # ---- constants / weights ----
            # x piece 0 first so stage 1 can begin ASAP; wq/wk/wv next;
            # wo + the mask are not needed until much later
            x_sb = [xp.tile([P, NT], BF16, tag=f"x{c}", name=f"x{c}")
                    for c in range(8)]
            # spread piece-0 chunk DMA issues across four engine queues:
            # a single queue issues one DMA descriptor per ~0.65us, which
            # would serialize the latency-critical first piece
            iss = [nc.sync, nc.sync, nc.sync, nc.scalar,
                   nc.scalar, nc.gpsimd, nc.gpsimd, nc.gpsimd]
            for c in range(8):
                iss[c].dma_start(x_sb[c][:, 0:512],
                                 xT_d.ap()[c * P:(c + 1) * P, 0:512])
            # mask + weights must land before the first attention group:
            # anything issued after the hoisted x DMAs queues behind
            # ~8 x 12.5us of per-queue transfers
            mask_sb = const.tile([P, 256], BF16, tag="mask")
            nc.sync.dma_start(mask_sb[:], mask_d.ap())
            w_sb = {}
            for name, dd in (("wq", wq_d), ("wk", wk_d), ("wv", wv_d)):
                t = const.tile([P, D], BF16, tag=name)
                nc.sync.dma_start(t[:], dd.ap())
                w_sb[name] = t
            wo = const.tile([P, D], BF16, tag="wo")
            nc.sync.dma_start(wo[:], wo_d.ap())
            w_sb["wo"] = wo
            for n in range(1, NP):
                cols = slice(n * 512, (n + 1) * 512)
                for c in range(8):
                    nc.sync.dma_start(x_sb[c][:, cols],
                                      xT_d.ap()[c * P:(c + 1) * P, cols])
            ident = const.tile([P, P], BF16, tag="ident")
            make_identity(nc, ident[:])

            # warm up the PE clock (the HAM throttle holds the PE at
            # half rate until it sees ~3.4us of sustained matmul
            # activity) while the first x DMAs are in flight
            warm = const.tile([P, 256], BF16, tag="warm")
            nc.vector.memset(warm[:], 0.0)
            wps = pp.tile([P, 512], F32, tag="p1", name="warmps")
            for i in range(28):
                nc.tensor.matmul(wps[:, 0:256], warm[:, 0:128], warm[:],
                                 start=True, stop=True)

            qT = qk.tile([P, NT], BF16, tag="qT")
            kT = qk.tile([P, NT], BF16, tag="kT")
            vT = qk.tile([P, NT], BF16, tag="vT")
            v_sb = qk.tile([P, (NT // P) * VCB], BF16, tag="v")
            nc.gpsimd.memset(v_sb[:], 1.0)
            ctxT = qk.tile([P, NT], BF16, tag="ctxT")

            # ---- stage 1 sub-units (half-size so they slot between
            # attention rounds without hogging the PE) ----
            def s1_proj(n, wname, dst):
                # self-contained unit: the psum tile opens and closes in
                # one pump so it never pins a p1 ring slot across rounds
                cols = slice(n * 512, (n + 1) * 512)
                w = w_sb[wname]
                ps = pp.tile([P, 512], F32, tag="p1", name=f"p1_{wname}_{n}")
                for cc in range(8):
                    nc.tensor.matmul(ps[:], w[:, cc * P:(cc + 1) * P],
                                     x_sb[cc][:, cols],
                                     start=(cc == 0), stop=(cc == 7))
                # GpSimd cannot read PSUM; ScalarE is reserved for exp,
                # so all stage-1 casts go to VectorE
                nc.vector.tensor_copy(dst[:, cols], ps[:])

            def s1_vtrans(n, half):
                # v -> token-major for 2 of the 4 blocks of this piece
                for t in range(4 * n + 2 * half, 4 * n + 2 * half + 2):
                    pst = pp.tile([P, P], BF16, tag="p1", name=f"ptr{t}")
                    nc.tensor.transpose(pst[:], vT[:, t * P:(t + 1) * P],
                                        ident[:])
                    # one 3D-AP copy places both heads' 64 feat cols
                    # (strides: head 65, feat 1), skipping the ones cols
                    dst3 = v_sb[:, t * VCB:(t + 1) * VCB].rearrange(
                        "p (h f) -> p h f", f=HD + 1)[:, :, 0:HD]
                    src3 = pst[:, :].rearrange("p (h f) -> p h f", f=HD)
                    nc.vector.tensor_copy(dst3, src3)

            q1 = []
            for n in range(NP):
                for wname, dst in (("wq", qT), ("wk", kT), ("wv", vT)):
                    q1.append((n, lambda n=n, w=wname, d=dst:
                               s1_proj(n, w, d)))
                q1 += [(n, lambda n=n: s1_vtrans(n, 0)),
                       (n, lambda n=n: s1_vtrans(n, 1))]

            i1 = 0
            done1 = -1

            def pump_one():
                # exactly one stage-1 unit: issuing whole pieces between
                # rounds gives them scheduler priority over later
                # attention rounds and serializes the pipeline.  The
                # tile_wait_until hint tells the scheduler when this
                # piece's x DMA actually lands (~10GB/s per queue, ~9
                # queues, ~6.5us per piece) so the frozen engine-queue
                # order doesn't put stage-1 work for late pieces ahead
                # of attention ops that are ready sooner.
                nonlocal i1, done1
                if i1 < len(q1):
                    n, fn = q1[i1]
                    with tc.tile_wait_until(0.009 + 0.0065 * n):
                        fn()
                    if i1 + 1 >= len(q1) or q1[i1 + 1][0] != n:
                        done1 = n
                    i1 += 1

            def pump_q1(need):
                while done1 < need and i1 < len(q1):
                    pump_one()

            # ---- stage 3: partial output projection; staging casts on
            # VectorE with an occasional ScalarE one (GpSimd cannot
            # read PSUM) ----
            veng = [nc.vector, nc.vector, nc.vector, nc.scalar]
            s3_ctr = [0]

            def s3_quarter(n, qtr, tail=False):
                cols = slice(n * 512, (n + 1) * 512)
                for f in range(qtr * 2, qtr * 2 + 2):
                    pso = pp.tile([P, 512], F32, tag="p1",
                                  name=f"p3_{f}_{n}")
                    nc.tensor.matmul(pso[:], wo[:, f * P:(f + 1) * P],
                                     ctxT[:, cols], start=True, stop=True)
                    st = stg.tile([P, 512], BF16, tag="st",
                                  name=f"st_{f}_{n}")
                    if tail:
                        # final drain: alternate V/Sc so one engine's
                        # serial copies don't pace the whole tail
                        eng = [nc.vector, nc.scalar][s3_ctr[0] % 2]
                    else:
                        eng = veng[s3_ctr[0] % 4]
                    if eng is nc.scalar:
                        eng.copy(st[:], pso[:])
                    else:
                        eng.tensor_copy(st[:], pso[:])
                    s3_ctr[0] += 1
                    nc.sync.dma_start(
                        out_d.ap()[f * P:(f + 1) * P, cols], st[:])

            def normalize(b, m, hl, cx):
                hbase = hl * HD
                qc0 = b * S + m * 512
                sm = misc.tile([1, 512], F32, tag="sm",
                               name=f"sm_{b}_{m}_{hl}")
                nc.vector.tensor_copy(sm[:], cx[HD:HD + 1, :])
                rc = misc.tile([1, 512], F32, tag="rc",
                               name=f"rc_{b}_{m}_{hl}")
                nc.vector.reciprocal_approx_fast(rc[:], sm[:])
                bc = misc.tile([HD, 512], F32, tag="bc",
                               name=f"bc_{b}_{m}_{hl}")
                nc.gpsimd.partition_broadcast(bc[:], rc[:])
                nc.vector.tensor_mul(
                    ctxT[hbase:hbase + HD, qc0:qc0 + 512],
                    cx[0:HD, :], bc[:])

            # ---- main loop: per (b, m) group, per k-block rounds ----
            s3q = []
            rnd = 0
            meng = [nc.vector, nc.vector]

            for b in range(B):
                for m in range(NM):
                    pump_q1(b * NM + m)
                    njs = 4 * m + 4
                    qc0 = b * S + m * 512
                    cx = [cxp.tile([HD + 1, 512], F32, tag="cx",
                                   name=f"cx_{b}_{m}_{hl}")
                          for hl in range(HPC)]

                    def ctx_pair(j, off, pt):
                        for hl in range(HPC):
                            vb = (b * (S // P) + j) * VCB + hl * (HD + 1)
                            nc.tensor.matmul(
                                cx[hl][:, off:512],
                                v_sb[:, vb:vb + HD + 1],
                                pt[:, hl * 512 + off:(hl + 1) * 512],
                                start=(j == 0), stop=(j == njs - 1))

                    pend = None
                    for j in range(njs):
                        off = 128 * max(0, j - 4 * m)
                        kc0 = b * S + j * P
                        scs = scp.tile([P, 1024], F32, tag="sc",
                                       name=f"sc_{b}_{m}_{j}")
                        # both heads' score matmuls issued back-to-back:
                        # 64-deep contractions at PE row groups 0 / 64
                        # run concurrently
                        for hl in range(HPC):
                            hb = hl * HD
                            nc.tensor.matmul(
                                scs[:, hl * 512 + off:(hl + 1) * 512],
                                kT[hb:hb + HD, kc0:kc0 + P],
                                qT[hb:hb + HD, qc0 + off:qc0 + 512],
                                start=True, stop=True,
                                tile_position=(hb, 0))
                        # one joint exp over both heads' unmasked region
                        pt = ptp.tile([P, 1024], BF16, tag="pt",
                                      name=f"pt_{b}_{m}_{j}")
                        if off:
                            nc.scalar.activation(
                                pt[:].rearrange("p (a c) -> p a c",
                                                a=2)[:, :, off:512],
                                scs[:].rearrange("p (a c) -> p a c",
                                                 a=2)[:, :, off:512],
                                Exp, scale=0.125)
                        else:
                            nc.scalar.activation(pt[:], scs[:], Exp,
                                                 scale=0.125)
                        if j >= 4 * m:
                            # diagonal block: 128-wide triangular window
                            # at [off, off+128) in each half
                            pt3 = pt[:].rearrange(
                                "p (a c) -> p a c", a=2)[:, :, off:off + 128]
                            m3 = mask_sb[:, :].rearrange(
                                "p (a c) -> p a c", a=2)
                            meng[j % 2].tensor_mul(pt3, pt3, m3)
                        # ctx of the PREVIOUS round issues after this
                        # round's scores so it can't pop between the
                        # two paired score matmuls
                        if pend is not None:
                            ctx_pair(*pend)
                        pend = (j, off, pt)
                        # stage-1 / stage-3 filler keeps the PE queue
                        # fed; keep a small s3 reserve so the PE stays
                        # warm through the end-of-kernel normalize chain
                        pump_one()
                        if len(s3q) > 2:
                            s3_quarter(*s3q.pop(0))
                        if len(s3q) > 5:
                            s3_quarter(*s3q.pop(0))
                        rnd += 1
                    ctx_pair(*pend)
                    for hl in range(HPC):
                        normalize(b, m, hl, cx[hl])
                    n = b * NM + m
                    s3q += [(n, 0), (n, 1), (n, 2), (n, 3)]
            pump_q1(NP)
            for n, qtr in s3q:
                s3_quarter(n, qtr, tail=True)
    nc.compile()
    return nc


def _get_nc():
    if "nc" not in _cache:
        _cache["nc"] = _build()
    return _cache["nc"]


def _bf16(a):
    return np.ascontiguousarray(a).astype(ml_dtypes.bfloat16)


def _prepare_in_maps(x, Wq, Wk, Wv, Wo):
    xT = _bf16(np.asarray(x, np.float32).reshape(NT, D).T)
    # triangular window mask (p <= c), duplicated for the two halves
    pp_ = np.arange(P)[:, None]
    cc = np.arange(P)[None, :]
    tri = (pp_ <= cc).astype(np.float32)
    mask = _bf16(np.concatenate([tri, tri], axis=1))

    def wlayout(Wslice):  # [128 feats, 1024 d] -> [p, cc*128+f]
        return _bf16(Wslice.reshape(P, 8, P).transpose(2, 1, 0)
                     .reshape(P, D))

    in_maps = []
    for c in range(NCORES):
        rows = slice(c * P, (c + 1) * P)
        in_maps.append({
            "xT": xT,
            "wq": wlayout(np.asarray(Wq, np.float32)[rows, :]),
            "wk": wlayout(np.asarray(Wk, np.float32)[rows, :]),
            "wv": wlayout(np.asarray(Wv, np.float32)[rows, :]),
            "wo": _bf16(np.asarray(Wo, np.float32)[:, rows].T),
            "mask": mask,
        })
    return in_maps


def _run(inputs, trace=False, tmpdir=None):
    from concourse.bass_utils import run_bass_kernel_spmd
    nc = _get_nc()
    in_maps = _prepare_in_maps(inputs["x"], inputs["Wq"], inputs["Wk"],
                               inputs["Wv"], inputs["Wo"])
    res = run_bass_kernel_spmd(nc, in_maps, core_ids=list(range(NCORES)),
                               trace=trace, tmpdir=tmpdir)
    acc = np.zeros((D, NT), np.float32)
    for r in res.results:
        acc += r["out"].astype(np.float32)
    out = acc.T.reshape(B, S, D) + np.asarray(inputs["bo"], np.float32)
    return out.astype(np.float32), res


def kernel(**inputs):
    out, _ = _run(inputs)
    return out


def kernel_traced(tmpdir=None, **inputs):
    out, res = _run(inputs, trace=True, tmpdir=tmpdir)
    return out, res


# revision 27
# speedup vs baseline: 1.6244x; 1.0250x over previous
"""Multi-head causal attention on 8 TRN2 NeuronCores.

B=2, S=2048, D=1024, H=16 heads, head_dim=64. Tensor-parallel over heads:
core c owns heads {2c, 2c+1}. Each core:
  stage 1 (per 512-token piece): qT/kT/vT = W_c @ x.T (feature-major,
           bf16 matmuls, fp32 psum), then v -> token-major via PE
           transpose with a ones column per head appended (gives the
           softmax denominator for free).
  stage 2: attention in scoresT (k-major) layout, joint over the two
           heads: for each 128-wide k-block j, both heads' score
           matmuls (64-deep contractions at PE row groups 0 and 64)
           are issued back-to-back into one shared [128,1024] PSUM
           tile so the hardware runs them concurrently.  One ScalarE
           exp covers both heads; causal handling is exact per block:
           fully-masked q-columns are never computed, and the single
           128-wide triangular window is masked with one small
           multiply (alternating VectorE/GpSimd).  ctxT' accumulation
           = [v|1].T @ pT per head; normalization by the ones-row sum
           via reciprocal_approx_fast + partition broadcast.
  stage 3: partial output projection split per head into 64-deep
           contractions at row groups 0/64 so consecutive f-blocks
           overlap on the PE; woven into the round stream.
A short burst of dummy matmuls at t=0 warms the PE HAM clock gate
during the initial x-DMA wait.  Host sums the 8 partial outputs and
adds the bias.
"""
import numpy as np
import ml_dtypes

B, S, D, H = 2, 2048, 1024, 16
HD = 64          # head dim
NT = B * S       # 4096 tokens
P = 128          # partitions
NCORES = 8
HPC = 2          # heads per core
NM = S // 512    # 4 q-pieces per batch
NP = NT // 512   # 8 token pieces overall
VCB = 2 * (HD + 1)   # 130: v block cols: h0 feats+1, h1 feats+1

_cache = {}


def _build():
    import concourse.bass as bass
    import concourse.mybir as mybir
    from concourse import bacc
    import concourse.tile as tile
    from concourse.masks import make_identity

    BF16 = mybir.dt.bfloat16
    F32 = mybir.dt.float32
    Exp = mybir.ActivationFunctionType.Exp

    nc = bacc.Bacc("TRN2", target_bir_lowering=False, debug=False,
                   num_devices=NCORES)

    xT_d = nc.dram_tensor("xT", [D, NT], BF16, kind="ExternalInput")
    wq_d = nc.dram_tensor("wq", [P, D], BF16, kind="ExternalInput")
    wk_d = nc.dram_tensor("wk", [P, D], BF16, kind="ExternalInput")
    wv_d = nc.dram_tensor("wv", [P, D], BF16, kind="ExternalInput")
    wo_d = nc.dram_tensor("wo", [P, D], BF16, kind="ExternalInput")
    mask_d = nc.dram_tensor("mask", [P, 256], BF16, kind="ExternalInput")
    out_d = nc.dram_tensor("out", [D, NT], BF16, kind="ExternalOutput")

    with tile.TileContext(nc) as tc:
        with tc.tile_pool(name="const", bufs=1) as const, \
             tc.tile_pool(name="xp", bufs=1) as xp, \
             tc.tile_pool(name="qk", bufs=1) as qk, \
             tc.tile_pool(name="misc", bufs=4) as misc, \
             tc.tile_pool(name="stg", bufs=6) as stg, \
             tc.tile_pool(name="pt", bufs=8) as ptp, \
             tc.tile_pool(name="pp", bufs=2, space="PSUM") as pp, \
             tc.tile_pool(name="sc", bufs=2, space="PSUM") as scp, \
             tc.tile_pool(name="cx", bufs=2, space="PSUM") as cxp:

            # ---- constants / weights ----
            # x piece 0 first so stage 1 can begin ASAP; wq/wk/wv next;
            # wo + the mask are not needed until much later
            x_sb = [xp.tile([P, NT], BF16, tag=f"x{c}", name=f"x{c}")
                    for c in range(8)]
            # spread piece-0 chunk DMA issues across four engine queues:
            # a single queue issues one DMA descriptor per ~0.65us, which
            # would serialize the latency-critical first piece
            iss = [nc.sync, nc.sync, nc.sync, nc.scalar,
                   nc.scalar, nc.gpsimd, nc.gpsimd, nc.gpsimd]
            for c in range(8):
                iss[c].dma_start(x_sb[c][:, 0:512],
                                 xT_d.ap()[c * P:(c + 1) * P, 0:512])
            # mask + weights must land before the first attention group:
            # anything issued after the hoisted x DMAs queues behind
            # ~8 x 12.5us of per-queue transfers
            mask_sb = const.tile([P, 256], BF16, tag="mask")
            nc.sync.dma_start(mask_sb[:], mask_d.ap())
            w_sb = {}
            for name, dd in (("wq", wq_d), ("wk", wk_d), ("wv", wv_d)):
                t = const.tile([P, D], BF16, tag=name)
                nc.sync.dma_start(t[:], dd.ap())
                w_sb[name] = t
            wo = const.tile([P, D], BF16, tag="wo")
            nc.sync.dma_start(wo[:], wo_d.ap())
            w_sb["wo"] = wo
            for n in range(1, NP):
                cols = slice(n * 512, (n + 1) * 512)
                for c in range(8):
                    nc.sync.dma_start(x_sb[c][:, cols],
                                      xT_d.ap()[c * P:(c + 1) * P, cols])
            ident = const.tile([P, P], BF16, tag="ident")
            make_identity(nc, ident[:])

            # warm up the PE clock (the HAM throttle holds the PE at
            # half rate until it sees ~3.4us of sustained matmul
            # activity) while the first x DMAs are in flight
            warm = const.tile([P, 256], BF16, tag="warm")
            nc.vector.memset(warm[:], 0.0)
            wps = pp.tile([P, 512], F32, tag="p1", name="warmps")
            for i in range(28):
                nc.tensor.matmul(wps[:, 0:256], warm[:, 0:128], warm[:],
                                 start=True, stop=True)

            qT = qk.tile([P, NT], BF16, tag="qT")
            kT = qk.tile([P, NT], BF16, tag="kT")
            vT = qk.tile([P, NT], BF16, tag="vT")
            v_sb = qk.tile([P, (NT // P) * VCB], BF16, tag="v")
            nc.gpsimd.memset(v_sb[:], 1.0)
            ctxT = qk.tile([P, NT], BF16, tag="ctxT")

            # ---- stage 1 sub-units (half-size so they slot between
            # attention rounds without hogging the PE) ----
            def s1_proj(n, wname, dst):
                # self-contained unit: the psum tile opens and closes in
                # one pump so it never pins a p1 ring slot across rounds
                cols = slice(n * 512, (n + 1) * 512)
                w = w_sb[wname]
                ps = pp.tile([P, 512], F32, tag="p1", name=f"p1_{wname}_{n}")
                for cc in range(8):
                    nc.tensor.matmul(ps[:], w[:, cc * P:(cc + 1) * P],
                                     x_sb[cc][:, cols],
                                     start=(cc == 0), stop=(cc == 7))
                # GpSimd cannot read PSUM; ScalarE is reserved for exp,
                # so all stage-1 casts go to VectorE
                nc.vector.tensor_copy(dst[:, cols], ps[:])

            def s1_vtrans(n, half):
                # v -> token-major for 2 of the 4 blocks of this piece
                for t in range(4 * n + 2 * half, 4 * n + 2 * half + 2):
                    pst = pp.tile([P, P], BF16, tag="p1", name=f"ptr{t}")
                    nc.tensor.transpose(pst[:], vT[:, t * P:(t + 1) * P],
                                        ident[:])
                    # one 3D-AP copy places both heads' 64 feat cols
                    # (strides: head 65, feat 1), skipping the ones cols
                    dst3 = v_sb[:, t * VCB:(t + 1) * VCB].rearrange(
                        "p (h f) -> p h f", f=HD + 1)[:, :, 0:HD]
                    src3 = pst[:, :].rearrange("p (h f) -> p h f", f=HD)
                    nc.vector.tensor_copy(dst3, src3)

            q1 = []
            for n in range(NP):
                for wname, dst in (("wq", qT), ("wk", kT), ("wv", vT)):
                    q1.append((n, lambda n=n, w=wname, d=dst:
                               s1_proj(n, w, d)))
                q1 += [(n, lambda n=n: s1_vtrans(n, 0)),
                       (n, lambda n=n: s1_vtrans(n, 1))]

            i1 = 0
            done1 = -1

            def pump_one():
                # exactly one stage-1 unit: issuing whole pieces between
                # rounds gives them scheduler priority over later
                # attention rounds and serializes the pipeline.  The
                # tile_wait_until hint tells the scheduler when this
                # piece's x DMA actually lands (~10GB/s per queue, ~9
                # queues, ~6.5us per piece) so the frozen engine-queue
                # order doesn't put stage-1 work for late pieces ahead
                # of attention ops that are ready sooner.
                nonlocal i1, done1
                if i1 < len(q1):
                    n, fn = q1[i1]
                    with tc.tile_wait_until(0.009 + 0.0065 * n):
                        fn()
                    if i1 + 1 >= len(q1) or q1[i1 + 1][0] != n:
                        done1 = n
                    i1 += 1

            def pump_q1(need):
                while done1 < need and i1 < len(q1):
                    pump_one()

            # ---- stage 3: partial output projection; staging casts on
            # VectorE with an occasional ScalarE one (GpSimd cannot
            # read PSUM) ----
            veng = [nc.vector, nc.vector, nc.vector, nc.scalar]
            s3_ctr = [0]

            def s3_quarter(n, qtr, tail=False):
                cols = slice(n * 512, (n + 1) * 512)
                for f in range(qtr * 2, qtr * 2 + 2):
                    pso = pp.tile([P, 512], F32, tag="p1",
                                  name=f"p3_{f}_{n}")
                    nc.tensor.matmul(pso[:], wo[:, f * P:(f + 1) * P],
                                     ctxT[:, cols], start=True, stop=True)
                    st = stg.tile([P, 512], BF16, tag="st",
                                  name=f"st_{f}_{n}")
                    if tail:
                        # final drain: alternate V/Sc so one engine's
                        # serial copies don't pace the whole tail
                        eng = [nc.vector, nc.scalar][s3_ctr[0] % 2]
                    else:
                        eng = veng[s3_ctr[0] % 4]
                    if eng is nc.scalar:
                        eng.copy(st[:], pso[:])
                    else:
                        eng.tensor_copy(st[:], pso[:])
                    s3_ctr[0] += 1
                    nc.sync.dma_start(
                        out_d.ap()[f * P:(f + 1) * P, cols], st[:])

            def normalize(b, m, hl, cx):
                hbase = hl * HD
                qc0 = b * S + m * 512
                sm = misc.tile([1, 512], F32, tag="sm",
                               name=f"sm_{b}_{m}_{hl}")
                nc.vector.tensor_copy(sm[:], cx[HD:HD + 1, :])
                rc = misc.tile([1, 512], F32, tag="rc",
                               name=f"rc_{b}_{m}_{hl}")
                nc.vector.reciprocal_approx_fast(rc[:], sm[:])
                bc = misc.tile([HD, 512], F32, tag="bc",
                               name=f"bc_{b}_{m}_{hl}")
                nc.gpsimd.partition_broadcast(bc[:], rc[:])
                nc.vector.tensor_mul(
                    ctxT[hbase:hbase + HD, qc0:qc0 + 512],
                    cx[0:HD, :], bc[:])

            # ---- main loop: per (b, m) group, per k-block rounds ----
            s3q = []
            rnd = 0
            meng = [nc.vector, nc.vector]

            for b in range(B):
                for m in range(NM):
                    pump_q1(b * NM + m)
                    njs = 4 * m + 4
                    qc0 = b * S + m * 512
                    cx = [cxp.tile([HD + 1, 512], F32, tag="cx",
                                   name=f"cx_{b}_{m}_{hl}")
                          for hl in range(HPC)]

                    def ctx_pair(j, off, pt):
                        for hl in range(HPC):
                            vb = (b * (S // P) + j) * VCB + hl * (HD + 1)
                            nc.tensor.matmul(
                                cx[hl][:, off:512],
                                v_sb[:, vb:vb + HD + 1],
                                pt[:, hl * 512 + off:(hl + 1) * 512],
                                start=(j == 0), stop=(j == njs - 1))

                    pend = None
                    for j in range(njs):
                        off = 128 * max(0, j - 4 * m)
                        kc0 = b * S + j * P
                        scs = scp.tile([P, 1024], F32, tag="sc",
                                       name=f"sc_{b}_{m}_{j}")
                        # both heads' score matmuls issued back-to-back:
                        # 64-deep contractions at PE row groups 0 / 64
                        # run concurrently.  high_priority keeps any
                        # instruction that becomes ready while the first
                        # half streams from popping between the two.
                        with tc.high_priority():
                            for hl in range(HPC):
                                hb = hl * HD
                                nc.tensor.matmul(
                                    scs[:, hl * 512 + off:(hl + 1) * 512],
                                    kT[hb:hb + HD, kc0:kc0 + P],
                                    qT[hb:hb + HD, qc0 + off:qc0 + 512],
                                    start=True, stop=True,
                                    tile_position=(hb, 0))
                        # one joint exp over both heads' unmasked region
                        pt = ptp.tile([P, 1024], BF16, tag="pt",
                                      name=f"pt_{b}_{m}_{j}")
                        if off:
                            nc.scalar.activation(
                                pt[:].rearrange("p (a c) -> p a c",
                                                a=2)[:, :, off:512],
                                scs[:].rearrange("p (a c) -> p a c",
                                                 a=2)[:, :, off:512],
                                Exp, scale=0.125)
                        else:
                            nc.scalar.activation(pt[:], scs[:], Exp,
                                                 scale=0.125)
                        if j >= 4 * m:
                            # diagonal block: 128-wide triangular window
                            # at [off, off+128) in each half
                            pt3 = pt[:].rearrange(
                                "p (a c) -> p a c", a=2)[:, :, off:off + 128]
                            m3 = mask_sb[:, :].rearrange(
                                "p (a c) -> p a c", a=2)
                            meng[j % 2].tensor_mul(pt3, pt3, m3)
                        # ctx of the PREVIOUS round issues after this
                        # round's scores so it can't pop between the
                        # two paired score matmuls
                        if pend is not None:
                            ctx_pair(*pend)
                        pend = (j, off, pt)
                        # stage-1 / stage-3 filler keeps the PE queue
                        # fed; keep a small s3 reserve so the PE stays
                        # warm through the end-of-kernel normalize chain
                        pump_one()
                        if len(s3q) > 2:
                            s3_quarter(*s3q.pop(0))
                        if len(s3q) > 5:
                            s3_quarter(*s3q.pop(0))
                        rnd += 1
                    ctx_pair(*pend)
                    for hl in range(HPC):
                        normalize(b, m, hl, cx[hl])
                    n = b * NM + m
                    s3q += [(n, 0), (n, 1), (n, 2), (n, 3)]
            pump_q1(NP)
            for n, qtr in s3q:
                s3_quarter(n, qtr, tail=True)
            # always-ready dummy matmuls issued after the drain (lowest
            # priority) soak up PE idle during the final normalize chain
            # so the HAM clock gate stays at full rate through the tail
            wps2 = scp.tile([P, 1024], F32, tag="sc", name="tailwarm")
            for i in range(22):
                nc.tensor.matmul(wps2[:, 0:512], warm[:, 0:128],
                                 qT[:, 0:512], start=True, stop=True)
    nc.compile()
    return nc


def _get_nc():
    if "nc" not in _cache:
        _cache["nc"] = _build()
    return _cache["nc"]


def _bf16(a):
    return np.ascontiguousarray(a).astype(ml_dtypes.bfloat16)


def _prepare_in_maps(x, Wq, Wk, Wv, Wo):
    xT = _bf16(np.asarray(x, np.float32).reshape(NT, D).T)
    # triangular window mask (p <= c), duplicated for the two halves
    pp_ = np.arange(P)[:, None]
    cc = np.arange(P)[None, :]
    tri = (pp_ <= cc).astype(np.float32)
    mask = _bf16(np.concatenate([tri, tri], axis=1))

    def wlayout(Wslice):  # [128 feats, 1024 d] -> [p, cc*128+f]
        return _bf16(Wslice.reshape(P, 8, P).transpose(2, 1, 0)
                     .reshape(P, D))

    in_maps = []
    for c in range(NCORES):
        rows = slice(c * P, (c + 1) * P)
        in_maps.append({
            "xT": xT,
            "wq": wlayout(np.asarray(Wq, np.float32)[rows, :]),
            "wk": wlayout(np.asarray(Wk, np.float32)[rows, :]),
            "wv": wlayout(np.asarray(Wv, np.float32)[rows, :]),
            "wo": _bf16(np.asarray(Wo, np.float32)[:, rows].T),
            "mask": mask,
        })
    return in_maps


def _run(inputs, trace=False, tmpdir=None):
    from concourse.bass_utils import run_bass_kernel_spmd
    nc = _get_nc()
    in_maps = _prepare_in_maps(inputs["x"], inputs["Wq"], inputs["Wk"],
                               inputs["Wv"], inputs["Wo"])
    res = run_bass_kernel_spmd(nc, in_maps, core_ids=list(range(NCORES)),
                               trace=trace, tmpdir=tmpdir)
    acc = np.zeros((D, NT), np.float32)
    for r in res.results:
        acc += r["out"].astype(np.float32)
    out = acc.T.reshape(B, S, D) + np.asarray(inputs["bo"], np.float32)
    return out.astype(np.float32), res


def kernel(**inputs):
    out, _ = _run(inputs)
    return out


def kernel_traced(tmpdir=None, **inputs):
    out, res = _run(inputs, trace=True, tmpdir=tmpdir)
    return out, res


# revision 38
# speedup vs baseline: 1.6963x; 1.0443x over previous
"""Multi-head causal attention on 8 TRN2 NeuronCores.

B=2, S=2048, D=1024, H=16 heads, head_dim=64. Tensor-parallel over heads:
core c owns heads {2c, 2c+1}. Each core:
  stage 1 (per 512-token piece): qT/kT/vT = W_c @ x.T (feature-major,
           bf16 matmuls, fp32 psum), then v -> token-major via PE
           transpose with a ones column per head appended (gives the
           softmax denominator for free).
  stage 2: attention in scoresT (k-major) layout, joint over the two
           heads: for each 128-wide k-block j, both heads' score
           matmuls (64-deep contractions at PE row groups 0 and 64)
           are issued back-to-back into one shared [128,1024] PSUM
           tile so the hardware runs them concurrently.  One ScalarE
           exp covers both heads; causal handling is exact per block:
           fully-masked q-columns are never computed, and the single
           128-wide triangular window is masked with one small
           multiply (alternating VectorE/GpSimd).  ctxT' accumulation
           = [v|1].T @ pT per head; normalization by the ones-row sum
           via reciprocal_approx_fast + partition broadcast.
  stage 3: partial output projection split per head into 64-deep
           contractions at row groups 0/64 so consecutive f-blocks
           overlap on the PE; woven into the round stream.
A short burst of dummy matmuls at t=0 warms the PE HAM clock gate
during the initial x-DMA wait.  Host sums the 8 partial outputs and
adds the bias.
"""
import numpy as np
import ml_dtypes

B, S, D, H = 2, 2048, 1024, 16
HD = 64          # head dim
NT = B * S       # 4096 tokens
P = 128          # partitions
NCORES = 8
HPC = 2          # heads per core
NM = S // 512    # 4 q-pieces per batch
NP = NT // 512   # 8 token pieces overall
VH = 72          # v cols per head per k-block: 64 feats + 1 one + 7 pad
                 # (pad keeps the DoubleRow k-tile stride 16B-aligned)
VCB = 2 * VH     # 144 v cols per k-block
VPB = 2 * VCB    # 288 v cols per k-block PAIR (DoubleRow k-tiles)

_cache = {}


def _build():
    import concourse.bass as bass
    import concourse.mybir as mybir
    from concourse import bacc
    import concourse.tile as tile
    from concourse.masks import make_identity

    BF16 = mybir.dt.bfloat16
    F32 = mybir.dt.float32
    FP8 = mybir.dt.float8e4
    DR = mybir.MatmulPerfMode.DoubleRow
    Exp = mybir.ActivationFunctionType.Exp

    nc = bacc.Bacc("TRN2", target_bir_lowering=False, debug=False,
                   num_devices=NCORES)

    xT_d = nc.dram_tensor("xT", [D, NT], BF16, kind="ExternalInput")
    wq_d = nc.dram_tensor("wq", [P, D], BF16, kind="ExternalInput")
    wk_d = nc.dram_tensor("wk", [P, D], BF16, kind="ExternalInput")
    wv_d = nc.dram_tensor("wv", [P, D], BF16, kind="ExternalInput")
    wo_d = nc.dram_tensor("wo", [P, D], BF16, kind="ExternalInput")
    mask_d = nc.dram_tensor("mask", [P, 256], BF16, kind="ExternalInput")
    out_d = nc.dram_tensor("out", [D, NT], BF16, kind="ExternalOutput")

    with tile.TileContext(nc) as tc:
        with tc.tile_pool(name="const", bufs=1) as const, \
             tc.tile_pool(name="xp", bufs=1) as xp, \
             tc.tile_pool(name="qk", bufs=1) as qk, \
             tc.tile_pool(name="misc", bufs=4) as misc, \
             tc.tile_pool(name="stg", bufs=6) as stg, \
             tc.tile_pool(name="pt", bufs=8) as ptp, \
             tc.tile_pool(name="pp", bufs=2, space="PSUM") as pp, \
             tc.tile_pool(name="sc", bufs=2, space="PSUM") as scp, \
             tc.tile_pool(name="cx", bufs=2, space="PSUM") as cxp:

            # ---- constants / weights ----
            # x piece 0 first so stage 1 can begin ASAP; wq/wk/wv next;
            # wo + the mask are not needed until much later
            x_sb = [xp.tile([P, NT], BF16, tag=f"x{c}", name=f"x{c}")
                    for c in range(8)]
            # spread piece-0 chunk DMA issues across four engine queues:
            # a single queue issues one DMA descriptor per ~0.65us, which
            # would serialize the latency-critical first piece
            iss = [nc.sync, nc.sync, nc.sync, nc.scalar,
                   nc.scalar, nc.gpsimd, nc.gpsimd, nc.gpsimd]
            for c in range(8):
                iss[c].dma_start(x_sb[c][:, 0:512],
                                 xT_d.ap()[c * P:(c + 1) * P, 0:512])
            # mask + weights must land before the first attention group:
            # anything issued after the hoisted x DMAs queues behind
            # ~8 x 12.5us of per-queue transfers
            mask_sb = const.tile([P, 256], BF16, tag="mask")
            nc.sync.dma_start(mask_sb[:], mask_d.ap())
            w_sb = {}
            for name, dd in (("wq", wq_d), ("wk", wk_d), ("wv", wv_d)):
                t = const.tile([P, D], BF16, tag=name)
                nc.sync.dma_start(t[:], dd.ap())
                w_sb[name] = t
            wo = const.tile([P, D], BF16, tag="wo")
            nc.sync.dma_start(wo[:], wo_d.ap())
            w_sb["wo"] = wo
            for n in range(1, NP):
                cols = slice(n * 512, (n + 1) * 512)
                for c in range(8):
                    nc.sync.dma_start(x_sb[c][:, cols],
                                      xT_d.ap()[c * P:(c + 1) * P, cols])
            ident = const.tile([P, P], BF16, tag="ident")
            make_identity(nc, ident[:])

            # warm up the PE clock (the HAM throttle holds the PE at
            # half rate until it sees ~3.4us of sustained matmul
            # activity) while the first x DMAs are in flight
            warm = const.tile([P, 256], BF16, tag="warm")
            nc.vector.memset(warm[:], 0.0)
            wps = pp.tile([P, 512], F32, tag="p1", name="warmps")
            for i in range(28):
                nc.tensor.matmul(wps[:, 0:256], warm[:, 0:128], warm[:],
                                 start=True, stop=True)

            qT = qk.tile([P, NT], BF16, tag="qT")
            kT = qk.tile([P, NT], BF16, tag="kT")
            vT = qk.tile([P, NT], BF16, tag="vT")
            # v token-major twice: fp8 (paired layout) for the
            # off-diagonal DoubleRow ctx matmuls, where softmax
            # averaging over >=512 positions washes out fp8 noise, and
            # bf16 for diagonal blocks, whose early tokens average few
            # positions and need full precision.  ones/pad cols = 1.0
            v_sb = qk.tile([P, (NT // P) * VCB], FP8, tag="v")
            nc.gpsimd.memset(v_sb[:], 1.0)
            v_sbd = qk.tile([P, (NT // P) * 2 * (HD + 1)], BF16, tag="vd")
            nc.gpsimd.memset(v_sbd[:], 1.0)
            ctxT = qk.tile([P, NT], BF16, tag="ctxT")

            # ---- stage 1 sub-units (half-size so they slot between
            # attention rounds without hogging the PE) ----
            def s1_proj(n, wname, dst):
                # self-contained unit: the psum tile opens and closes in
                # one pump so it never pins a p1 ring slot across rounds
                cols = slice(n * 512, (n + 1) * 512)
                w = w_sb[wname]
                ps = pp.tile([P, 512], F32, tag="p1", name=f"p1_{wname}_{n}")
                for cc in range(8):
                    nc.tensor.matmul(ps[:], w[:, cc * P:(cc + 1) * P],
                                     x_sb[cc][:, cols],
                                     start=(cc == 0), stop=(cc == 7))
                # GpSimd cannot read PSUM; ScalarE is reserved for exp,
                # so all stage-1 casts go to VectorE
                nc.vector.tensor_copy(dst[:, cols], ps[:])

            def s1_vtrans(n, half):
                # v -> token-major for 2 of the 4 blocks of this piece
                for t in range(4 * n + 2 * half, 4 * n + 2 * half + 2):
                    pst = pp.tile([P, P], BF16, tag="p1", name=f"ptr{t}")
                    nc.tensor.transpose(pst[:], vT[:, t * P:(t + 1) * P],
                                        ident[:])
                    # 3D-AP copies place both heads' 64 feat cols,
                    # skipping ones/pad cols, into both v layouts
                    src3 = pst[:, :].rearrange("p (h f) -> p h f", f=HD)
                    dst3 = v_sb[:, t * VCB:(t + 1) * VCB].rearrange(
                        "p (h f) -> p h f", f=VH)[:, :, 0:HD]
                    nc.vector.tensor_copy(dst3, src3)
                    dstd = v_sbd[:, t * 2 * (HD + 1):(t + 1) * 2 * (HD + 1)
                                 ].rearrange("p (h f) -> p h f",
                                             f=HD + 1)[:, :, 0:HD]
                    nc.vector.tensor_copy(dstd, src3)

            q1 = []
            for n in range(NP):
                for wname, dst in (("wq", qT), ("wk", kT), ("wv", vT)):
                    q1.append((n, lambda n=n, w=wname, d=dst:
                               s1_proj(n, w, d)))
                q1 += [(n, lambda n=n: s1_vtrans(n, 0)),
                       (n, lambda n=n: s1_vtrans(n, 1))]

            i1 = 0
            done1 = -1

            def pump_one():
                # exactly one stage-1 unit: issuing whole pieces between
                # rounds gives them scheduler priority over later
                # attention rounds and serializes the pipeline.  The
                # tile_wait_until hint tells the scheduler when this
                # piece's x DMA actually lands (~10GB/s per queue, ~9
                # queues, ~6.5us per piece) so the frozen engine-queue
                # order doesn't put stage-1 work for late pieces ahead
                # of attention ops that are ready sooner.
                nonlocal i1, done1
                if i1 < len(q1):
                    n, fn = q1[i1]
                    with tc.tile_wait_until(0.009 + 0.0065 * n):
                        fn()
                    if i1 + 1 >= len(q1) or q1[i1 + 1][0] != n:
                        done1 = n
                    i1 += 1

            def pump_q1(need):
                while done1 < need and i1 < len(q1):
                    pump_one()

            # ---- stage 3: partial output projection; staging casts on
            # VectorE with an occasional ScalarE one (GpSimd cannot
            # read PSUM) ----
            veng = [nc.vector, nc.vector, nc.vector, nc.scalar]
            s3_ctr = [0]

            def s3_quarter(n, qtr, tail=False):
                cols = slice(n * 512, (n + 1) * 512)
                for f in range(qtr * 2, qtr * 2 + 2):
                    pso = pp.tile([P, 512], F32, tag="p1",
                                  name=f"p3_{f}_{n}")
                    nc.tensor.matmul(pso[:], wo[:, f * P:(f + 1) * P],
                                     ctxT[:, cols], start=True, stop=True)
                    st = stg.tile([P, 512], BF16, tag="st",
                                  name=f"st_{f}_{n}")
                    if tail:
                        # final drain: alternate V/Sc so one engine's
                        # serial copies don't pace the whole tail
                        eng = [nc.vector, nc.scalar][s3_ctr[0] % 2]
                    else:
                        eng = veng[s3_ctr[0] % 4]
                    if eng is nc.scalar:
                        eng.copy(st[:], pso[:])
                    else:
                        eng.tensor_copy(st[:], pso[:])
                    s3_ctr[0] += 1
                    nc.sync.dma_start(
                        out_d.ap()[f * P:(f + 1) * P, cols], st[:])

            def normalize(b, m, hl, cx):
                hbase = hl * HD
                qc0 = b * S + m * 512
                sm = misc.tile([1, 512], F32, tag="sm",
                               name=f"sm_{b}_{m}_{hl}")
                nc.vector.tensor_copy(sm[:], cx[HD:HD + 1, :])
                rc = misc.tile([1, 512], F32, tag="rc",
                               name=f"rc_{b}_{m}_{hl}")
                nc.vector.reciprocal_approx_fast(rc[:], sm[:])
                bc = misc.tile([HD, 512], F32, tag="bc",
                               name=f"bc_{b}_{m}_{hl}")
                nc.gpsimd.partition_broadcast(bc[:], rc[:])
                nc.vector.tensor_mul(
                    ctxT[hbase:hbase + HD, qc0:qc0 + 512],
                    cx[0:HD, :], bc[:])

            # ---- main loop: per (b, m) group, per k-block rounds ----
            s3q = []
            rnd = 0
            meng = [nc.vector, nc.vector]

            for b in range(B):
                for m in range(NM):
                    pump_q1(b * NM + m)
                    njs = 4 * m + 4
                    qc0 = b * S + m * 512
                    cx = [cxp.tile([HD + 1, 512], F32, tag="cx",
                                   name=f"cx_{b}_{m}_{hl}")
                          for hl in range(HPC)]

                    def ctx_dr(jodd, pt2):
                        # off-diagonal pair: one fp8 DoubleRow matmul
                        # contracts k-blocks jodd-1 and jodd at once
                        vb = (b * (S // P) + jodd - 1) * VCB
                        v3 = v_sb[:, vb:vb + VPB].rearrange(
                            "p (t c) -> p t c", t=2)
                        p3 = pt2[:].rearrange("p (t x) -> p t x", t=2)
                        for hl in range(HPC):
                            nc.tensor.matmul(
                                cx[hl][:, 0:512],
                                v3[:, :, hl * VH:hl * VH + HD + 1],
                                p3[:, :, hl * 512:(hl + 1) * 512],
                                start=(jodd == 1), stop=False,
                                perf_mode=DR)

                    def ctx_bf(j, off, pt):
                        # diagonal block: bf16 per-block matmul keeps
                        # full precision where few positions average
                        for hl in range(HPC):
                            vb = (b * (S // P) + j) * 2 * (HD + 1) \
                                + hl * (HD + 1)
                            nc.tensor.matmul(
                                cx[hl][:, off:512],
                                v_sbd[:, vb:vb + HD + 1],
                                pt[:, hl * 512 + off:(hl + 1) * 512],
                                start=(j == 0), stop=(j == njs - 1))

                    pend = None
                    pt2 = None
                    for j in range(njs):
                        off = 128 * max(0, j - 4 * m)
                        kc0 = b * S + j * P
                        scs = scp.tile([P, 1024], F32, tag="sc",
                                       name=f"sc_{b}_{m}_{j}")
                        # both heads' score matmuls issued back-to-back:
                        # 64-deep contractions at PE row groups 0 / 64
                        # run concurrently.  high_priority keeps any
                        # instruction that becomes ready while the first
                        # half streams from popping between the two.
                        with tc.high_priority():
                            for hl in range(HPC):
                                hb = hl * HD
                                nc.tensor.matmul(
                                    scs[:, hl * 512 + off:(hl + 1) * 512],
                                    kT[hb:hb + HD, kc0:kc0 + P],
                                    qT[hb:hb + HD, qc0 + off:qc0 + 512],
                                    start=True, stop=True,
                                    tile_position=(hb, 0))
                        diag = j >= 4 * m
                        if not diag:
                            if j % 2 == 0:
                                # fp8 pt tile shared by the off-diag
                                # pair (j, j+1)
                                pt2 = ptp.tile([P, 2048], FP8, tag="ptf",
                                               name=f"ptf_{b}_{m}_{j}")
                            phalf = pt2[:, (j % 2) * 1024:
                                        (j % 2 + 1) * 1024]
                            nc.scalar.activation(phalf, scs[:], Exp,
                                                 scale=0.125)
                        else:
                            pt = ptp.tile([P, 1024], BF16, tag="ptd",
                                          name=f"ptd_{b}_{m}_{j}")
                            if off:
                                nc.scalar.activation(
                                    pt[:].rearrange("p (a c) -> p a c",
                                                    a=2)[:, :, off:512],
                                    scs[:].rearrange("p (a c) -> p a c",
                                                     a=2)[:, :, off:512],
                                    Exp, scale=0.125)
                            else:
                                nc.scalar.activation(pt[:], scs[:], Exp,
                                                     scale=0.125)
                            # 128-wide triangular window at
                            # [off, off+128) in each half
                            pt3 = pt[:].rearrange(
                                "p (a c) -> p a c", a=2)[:, :, off:off + 128]
                            m3 = mask_sb[:, :].rearrange(
                                "p (a c) -> p a c", a=2)
                            meng[j % 2].tensor_mul(pt3, pt3, m3)
                        # ctx of the PREVIOUS round issues after this
                        # round's scores so it can't pop between the
                        # two paired score matmuls
                        if pend is not None:
                            ctx_pair, args = pend
                            ctx_pair(*args)
                            pend = None
                        if diag:
                            pend = (ctx_bf, (j, off, pt))
                        elif j % 2 == 1:
                            pend = (ctx_dr, (j, pt2))
                        # stage-1 / stage-3 filler keeps the PE queue
                        # fed; keep a small s3 reserve so the PE stays
                        # warm through the end-of-kernel normalize chain
                        pump_one()
                        if len(s3q) > 2:
                            s3_quarter(*s3q.pop(0))
                        if len(s3q) > 5:
                            s3_quarter(*s3q.pop(0))
                        rnd += 1
                    fn, args = pend
                    fn(*args)
                    for hl in range(HPC):
                        normalize(b, m, hl, cx[hl])
                    n = b * NM + m
                    s3q += [(n, 0), (n, 1), (n, 2), (n, 3)]
            pump_q1(NP)
            for n, qtr in s3q:
                s3_quarter(n, qtr, tail=True)
            # always-ready dummy matmuls issued after the drain (lowest
            # priority) soak up PE idle during the final normalize chain
            # so the HAM clock gate stays at full rate through the tail
            wps2 = scp.tile([P, 1024], F32, tag="sc", name="tailwarm")
            for i in range(22):
                nc.tensor.matmul(wps2[:, 0:512], warm[:, 0:128],
                                 qT[:, 0:512], start=True, stop=True)
    nc.compile()
    return nc


def _get_nc():
    if "nc" not in _cache:
        _cache["nc"] = _build()
    return _cache["nc"]


def _bf16(a):
    return np.ascontiguousarray(a).astype(ml_dtypes.bfloat16)


def _prepare_in_maps(x, Wq, Wk, Wv, Wo):
    xT = _bf16(np.asarray(x, np.float32).reshape(NT, D).T)
    # triangular window mask (p <= c), duplicated for the two halves
    pp_ = np.arange(P)[:, None]
    cc = np.arange(P)[None, :]
    tri = (pp_ <= cc).astype(np.float32)
    mask = _bf16(np.concatenate([tri, tri], axis=1))

    def wlayout(Wslice):  # [128 feats, 1024 d] -> [p, cc*128+f]
        return _bf16(Wslice.reshape(P, 8, P).transpose(2, 1, 0)
                     .reshape(P, D))

    in_maps = []
    for c in range(NCORES):
        rows = slice(c * P, (c + 1) * P)
        in_maps.append({
            "xT": xT,
            "wq": wlayout(np.asarray(Wq, np.float32)[rows, :]),
            "wk": wlayout(np.asarray(Wk, np.float32)[rows, :]),
            "wv": wlayout(np.asarray(Wv, np.float32)[rows, :]),
            "wo": _bf16(np.asarray(Wo, np.float32)[:, rows].T),
            "mask": mask,
        })
    return in_maps


def _run(inputs, trace=False, tmpdir=None):
    from concourse.bass_utils import run_bass_kernel_spmd
    nc = _get_nc()
    in_maps = _prepare_in_maps(inputs["x"], inputs["Wq"], inputs["Wk"],
                               inputs["Wv"], inputs["Wo"])
    res = run_bass_kernel_spmd(nc, in_maps, core_ids=list(range(NCORES)),
                               trace=trace, tmpdir=tmpdir)
    acc = np.zeros((D, NT), np.float32)
    for r in res.results:
        acc += r["out"].astype(np.float32)
    out = acc.T.reshape(B, S, D) + np.asarray(inputs["bo"], np.float32)
    return out.astype(np.float32), res


def kernel(**inputs):
    out, _ = _run(inputs)
    return out


def kernel_traced(tmpdir=None, **inputs):
    out, res = _run(inputs, trace=True, tmpdir=tmpdir)
    return out, res
